# revision 58
# baseline (speedup 1.0000x reference)
"""GAT+JumpingKnowledge GNN kernel for 8 Trainium2 NeuronCores.

Sharding: nodes are assigned to cores round-robin by global in-degree rank
(6250/core; profiles match across cores so the SPMD round maxima stay
tight).  Each core, per layer:
  - projects its own nodes' features h = x @ [W | W@a_src | W@a_dst] (f16)
  - stages packed 132B table rows [64 x f16 h | f32 alpha_src] and
    AllGathers them in four rank-range splits (a1/a2/b1/b2) that fire as
    soon as their projections land; the b2 trigger+fill are deferred into
    the next layer's gather stream so the in-order GpSimd sequencer never
    stalls on them
  - gathers, per dst-node "slot grid" (nodes on partitions, incoming-edge
    rounds on the free dim), the src rows of its edges with a custom
    SBUF-source dma_gather.  Descriptor GENERATION on the Q7 is the
    bottleneck (~2.2ns/slot, serialized), so gathers are fused into
    1024-descriptor chunks that span blocks; the A-class stream leads the
    B-class+compute stream by LOOKAHEAD blocks through per-class SBUF
    rings so queues stay fed across layer boundaries
  - computes the edge softmax (no max subtraction; logit range ~[-7, 7])
    and the weighted aggregation with a DVE multiply + in-place f16
    halving-tree reduction (contiguous adds, R_CAP-round segments)
  - self-loops never touch the gather path: their contribution is computed
    locally from per-block alpha_src/alpha_dst and the kept h copy
Final JK-max + output projection happen on the owned nodes; the host
reassembles the full [50000, 40] output via the node assignment.
"""

import numpy as np

# --- problem constants (hardcoded per harness contract) ---
N = 50000
E = 1600000
F_IN = 128
H = 64
L = 3
OUT = 40
NEG_SLOPE = 0.2
NC = 8
NPC_REAL = N // NC          # 6250 real nodes per core
BLOCKS = 49                 # ceil(6250/128)
NPC = BLOCKS * 128          # 6272 padded nodes per core
BLOCKS_A = 25               # blocks in table half A (local rows [0, 3200))
ROWS_A = BLOCKS_A * 128     # 3200
ROWS_B = NPC - ROWS_A       # 3072
TAB_A = NC * ROWS_A         # 25600 rows in gathered half-A table
TAB_B = NC * ROWS_B         # 24576
PAD_A = ROWS_A - 1          # local pad row 3199 (half A dummy)
DUMMY_A = PAD_A             # core 0's pad row in A-table coords
DUMMY_B = 6251 - ROWS_A     # core 0's pad row 6251 in B-table coords
ELEM = 33                   # gathered element: 33 f32 = 132B (64 f16 h + f32 alpha)
SB_BLOCKS = 1               # blocks per superblock (gather granularity)
ALPHA_NEG = -1.0e30


# ---------------------------------------------------------------------------
# Host-side graph preprocessing
# ---------------------------------------------------------------------------

def _fill_grid(Rn, slot_p, rows_vals, dummy):
    """Grid [Rn, 128] in i=r*128+p order; node p's edges fill rounds 0..k-1."""
    grid = np.full((int(Rn), 128), dummy, np.int64)
    o = np.argsort(slot_p, kind="stable")
    ps = slot_p[o]
    rv = rows_vals[o]
    first = np.searchsorted(ps, np.arange(128), side="left")
    ranks = np.arange(len(ps)) - first[ps]
    grid[ranks, ps] = rv
    return grid.reshape(-1)


def _preprocess(edge_index):
    """Self-loops are handled locally on-device (never gathered).  Nodes are
    assigned to cores round-robin by global in-degree rank so every core's
    per-block degree profile matches (tight cross-core round maxima).  Within
    each core the top-3199 nodes are class A (table rows [0, 3199)), the rest
    class B; each class is sorted by (max(ka,kb), ka+kb) desc into its rows."""
    src = edge_index[0].astype(np.int64)
    dst = edge_index[1].astype(np.int64)

    deg = np.bincount(dst, minlength=N)
    order = np.argsort(-deg, kind="stable")
    core_of = np.empty(N, np.int64)
    core_of[order] = np.arange(N) % NC
    lrank = np.empty(N, np.int64)
    lrank[order] = np.arange(N) // NC
    is_a_node = lrank < PAD_A

    sA = is_a_node[src]
    ka_n = np.zeros(N, np.int64)
    np.add.at(ka_n, dst[sA], 1)
    kb_n = np.zeros(N, np.int64)
    np.add.at(kb_n, dst[~sA], 1)

    nodes_of = np.full((NC, NPC), -1, np.int64)   # row -> global node id
    row_of = np.full(N, -1, np.int64)             # node -> row in its core
    RL = np.zeros(BLOCKS, np.int64)
    RH = np.zeros(BLOCKS, np.int64)
    for c in range(NC):
        nodes = np.where(core_of == c)[0]
        for cls, row0 in ((True, 0), (False, ROWS_A)):
            ids = nodes[is_a_node[nodes] == cls]
            o = ids[np.lexsort((-(ka_n[ids] + kb_n[ids]),
                                -np.maximum(ka_n[ids], kb_n[ids])))]
            nodes_of[c, row0:row0 + len(o)] = o
            row_of[o] = row0 + np.arange(len(o))
        kar = np.where(nodes_of[c] >= 0, ka_n[np.maximum(nodes_of[c], 0)], 0)
        kbr = np.where(nodes_of[c] >= 0, kb_n[np.maximum(nodes_of[c], 0)], 0)
        RL = np.maximum(RL, kar.reshape(BLOCKS, 128).max(axis=1))
        RH = np.maximum(RH, kbr.reshape(BLOCKS, 128).max(axis=1))

    src_core = core_of[src]
    src_row = row_of[src]
    e_is_a = src_row < ROWS_A
    rows_a_all = src_core * ROWS_A + src_row
    rows_b_all = src_core * ROWS_B + (src_row - ROWS_A)
    slot_all = row_of[dst]

    idx_a_cores, idx_b_cores = [], []
    for c in range(NC):
        m = core_of[dst] == c
        slot_of = slot_all[m]
        is_a = e_is_a[m]
        rows_a = rows_a_all[m]
        rows_b = rows_b_all[m]
        la, lb = [], []
        for bidx in range(BLOCKS):
            base = bidx * 128
            in_blk = (slot_of >= base) & (slot_of < base + 128)
            sel = in_blk & is_a
            la.append(_fill_grid(RL[bidx], slot_of[sel] - base, rows_a[sel],
                                 DUMMY_A))
            sel = in_blk & ~is_a
            lb.append(_fill_grid(RH[bidx], slot_of[sel] - base, rows_b[sel],
                                 DUMMY_B))
        idx_a_cores.append(np.concatenate(la).astype(np.int16))
        idx_b_cores.append(np.concatenate(lb).astype(np.int16))

    return nodes_of, idx_a_cores, idx_b_cores, RL, RH


def _alpha_mask():
    """[128, BLOCKS] f32: -1e30 on pad rows (3199, 6251..6271), else 0."""
    mask = np.zeros((NPC,), np.float32)
    mask[PAD_A] = ALPHA_NEG
    mask[6251:] = ALPHA_NEG
    return np.ascontiguousarray(mask.reshape(BLOCKS, 128).T)


def _wrap_idx(flat):
    """[num] -> [128, num//16] wrapped (i%16, i//16), replicated to 128 parts."""
    num = len(flat)
    assert num % 16 == 0
    w = flat.reshape(num // 16, 16).T
    return np.ascontiguousarray(np.tile(w, (8, 1))).astype(np.int16)


# ---------------------------------------------------------------------------
# Device kernel builder
# ---------------------------------------------------------------------------

def _gather_sbuf(nc, out_ap, in_ap, idxs_ap, num_idxs, elem_size, queue_num,
                 reg=None):
    """Non-transpose dma_gather from an SBUF-resident table.

    Mirrors concourse.bass.BassGpSimd.dma_gather minus its "SBUF source
    implies transpose" restriction: the Q7 ucode's SBUF addressing branch
    (token = idx % 128 -> partition, rank = idx // 128 -> free-dim stripe)
    is independent of the transpose flag, and the non-transpose RX side
    writes the standard [128, num_idxs/128, elem] grid layout.
    """
    import concourse.mybir as mybir

    eng = nc.gpsimd
    elem_bytes = elem_size * mybir.dt.size(in_ap.dtype)
    return eng.add_instruction(
        mybir.InstDMAGatherAnt(
            name=eng.bass.get_next_instruction_name(),
            ins=[
                eng.lower_ap(in_ap),
                eng.lower_ap(idxs_ap),
                eng.lower_val_access(reg if reg is not None
                                     else eng.to_reg(num_idxs)),
            ],
            outs=[eng.lower_ap(out_ap)],
            transpose=False,
            num_idxs=num_idxs,
            elem_size=elem_size,
            stride_bytes_256=0,
            gen_mode=0,
            single_packet=True,
            queue_num=queue_num,
            sbuf_tokens_per_rank=128,
            sbuf_free_dim_per_rank=elem_bytes,
            sbuf_free_dim_pad_per_rank=0,
            sbuf_byte_offset=0,
        )
    )


def _build(nc, RL, RH, n_idx_a, n_idx_b):
    import contextlib

    import concourse.mybir as mybir
    import concourse.tile as tile
    from concourse import library_config
    from concourse.masks import make_identity

    f32 = mybir.dt.float32
    f16 = mybir.dt.float16
    AF = mybir.ActivationFunctionType
    ALU = mybir.AluOpType

    # --- I/O ---
    # x is pre-transposed on the host so layer-0 projection feeds the PE
    # stationary operand straight from DRAM (no per-block PE transpose).
    x_in = nc.dram_tensor("xT_own", [F_IN, NPC], f16, kind="ExternalInput").ap()
    w1_in = nc.dram_tensor("w1", [F_IN, H], f32, kind="ExternalInput").ap()
    w23_in = nc.dram_tensor("w23", [L - 1, H, H], f32, kind="ExternalInput").ap()
    asrc_in = nc.dram_tensor("asrc", [L, H], f32, kind="ExternalInput").ap()
    adst_in = nc.dram_tensor("adst", [L, H], f32, kind="ExternalInput").ap()
    bias_in = nc.dram_tensor("bias", [L, H], f32, kind="ExternalInput").ap()
    wout_in = nc.dram_tensor("wout", [H, OUT], f32, kind="ExternalInput").ap()
    bout_in = nc.dram_tensor("bout", [1, OUT], f32, kind="ExternalInput").ap()
    idxa_in = nc.dram_tensor("idx_a", [128, n_idx_a // 16], mybir.dt.int16,
                             kind="ExternalInput").ap()
    idxb_in = nc.dram_tensor("idx_b", [128, n_idx_b // 16], mybir.dt.int16,
                             kind="ExternalInput").ap()
    amask_in = nc.dram_tensor("alpha_mask", [128, BLOCKS], f32,
                              kind="ExternalInput").ap()
    out_t = nc.dram_tensor("y", [NPC, OUT], f32, kind="ExternalOutput").ap()

    # --- internal DRAM ---
    # Compact partition-major tables: core-local row r lives at
    # [r % 128, r // 128, :], so the post-AllGather DRAM->SBUF fill runs at
    # line rate (one big descriptor per (core, partition)).
    BLOCKS_B = BLOCKS - BLOCKS_A
    # The table halves are AllGathered in four rank-range splits so each
    # collective fires as soon as its projections land; only the last split's
    # (short) chain remains exposed at a layer boundary.
    # (class, name, rank lo, rank hi) -- ranks are within the class table.
    TAB_SPLITS = [("a", "a1", 0, 10), ("a", "a2", 10, BLOCKS_A),
                  ("b", "b1", 0, 12), ("b", "b2", 12, BLOCKS_B)]
    tab_own = {}
    tab_full = {}
    for cls, nm, lo, hi in TAB_SPLITS:
        tab_own[nm] = nc.dram_tensor(f"tab_own_{nm}", [128, hi - lo, ELEM],
                                     f32, kind="Internal").ap()
        tab_full[nm] = nc.dram_tensor(f"tab_full_{nm}",
                                      [NC, 128, hi - lo, ELEM], f32,
                                      kind="Internal",
                                      addr_space="Shared").ap()

    R_TOT = [int(RL[b] + RH[b]) for b in range(BLOCKS)]
    R_MAX = max(R_TOT)
    CHUNK = 1024         # gather chunk (descriptors per SWDGE instruction)
    LOOKAHEAD = 10       # blocks the A-class gather stream leads by
    R_CAP = 38           # max rounds per multiply+tree segment (wt scratch)
    GA, GB = 5, 6        # row-store group sizes (25 = 5*5, 24 = 4*6)
    PGRP = 6             # next-layer projection burst size
    PGRP_Y = 8           # last-layer output projection burst size

    # Per-block queue: all of a block's gather chunks share one queue so a
    # blocked chunk (grid-pool WAR, pending fill) parks only its own queue.
    # Greedy-balance block costs (~R_TOT) across the 4 SWDGE queues.
    qload = [0] * 4
    QMAP = [0] * BLOCKS
    for b in sorted(range(BLOCKS), key=lambda x: -R_TOT[x]):
        q = min(range(4), key=lambda i: qload[i])
        QMAP[b] = q
        qload[q] += R_TOT[b]

    warm_in = nc.dram_tensor("cc_warm_in", [1, 16], f32, kind="Internal").ap()
    warm_out = nc.dram_tensor("cc_warm_out", [NC, 1, 16], f32, kind="Internal",
                              addr_space="Shared").ap()

    with tile.TileContext(nc) as tc:
        nc.gpsimd.load_library(library_config.mlp)

        with contextlib.ExitStack() as ctx:
            const = ctx.enter_context(tc.tile_pool(name="const", bufs=1))
            psum = ctx.enter_context(tc.tile_pool(name="psum", bufs=4, space="PSUM"))
            work = ctx.enter_context(tc.tile_pool(name="work", bufs=3))
            small = ctx.enter_context(tc.tile_pool(name="small", bufs=4))

            nc.gpsimd.collective_compute(
                "AllGather", ALU.bypass, replica_groups=[list(range(NC))],
                ins=[warm_in.opt()], outs=[warm_out.opt()])
            ident = const.tile([128, 128], f32, tag="ident")
            make_identity(nc, ident[:])
            ident16 = const.tile([128, 128], f16, tag="ident16")
            make_identity(nc, ident16[:])
            ones_row = const.tile([1, 128], f32, tag="ones")
            nc.vector.memset(ones_row[:], 1.0)
            idxa_sb = const.tile([128, n_idx_a // 16], mybir.dt.int16, tag="idxa")
            nc.sync.dma_start(idxa_sb[:], idxa_in[:])
            idxb_sb = const.tile([128, n_idx_b // 16], mybir.dt.int16, tag="idxb")
            nc.sync.dma_start(idxb_sb[:], idxb_in[:])
            x_buf = const.tile([128, BLOCKS * H], f16, tag="xbuf")
            jk_buf = const.tile([128, BLOCKS * H], f16, tag="jkbuf")
            sb_tab_a = const.tile([128, NC * BLOCKS_A * ELEM], f32, tag="taba")
            sb_tab_b = const.tile([128, NC * BLOCKS_B * ELEM], f32, tag="tabb")
            sb_ta3 = sb_tab_a[:].rearrange("p (k e) -> p k e", e=ELEM)
            sb_tb3 = sb_tab_b[:].rearrange("p (k e) -> p k e", e=ELEM)
            alphad = const.tile([128, BLOCKS], f32, tag="alphad")
            alphas = const.tile([128, BLOCKS], f32, tag="alphas")
            h_buf = const.tile([128, BLOCKS * H], f16, tag="hbuf")
            amask = const.tile([128, BLOCKS], f32, tag="amask")
            nc.sync.dma_start(amask[:], amask_in[:])
            ebias = const.tile([128, 1], f32, tag="ebias")
            nc.vector.memset(ebias[:], -2.772588722239781)

            self_q = [0]
            reg_full = [None]
            stage_state = {}

            def prep_weights(layer):
                """[W | W@a_src | W@a_dst] + bias broadcast tile for layer."""
                F = F_IN if layer == 0 else H
                w_ap = w1_in if layer == 0 else w23_in[layer - 1]
                waug = small.tile([128, H + 2], f32, tag="waug")
                nc.sync.dma_start(waug[:F, 0:H], w_ap)
                wt_ps = psum.tile([H, 128], f32, tag="ps_t")
                nc.tensor.transpose(wt_ps[:, :F], waug[:F, 0:H], ident[:F, :F])
                wt_sb = small.tile([H, 128], f32, tag="wtsb")
                nc.scalar.copy(wt_sb[:, :F], wt_ps[:, :F])
                a_cols = small.tile([H, 2], f32, tag="acols")
                nc.sync.dma_start(a_cols[:, 0:1], asrc_in[layer, :, None])
                nc.sync.dma_start(a_cols[:, 1:2], adst_in[layer, :, None])
                va_ps = psum.tile([128, 2], f32, tag="ps_m")
                nc.tensor.matmul(va_ps[:F, :], wt_sb[:, :F], a_cols[:],
                                 start=True, stop=True)
                nc.vector.tensor_copy(waug[:F, H:H + 2], va_ps[:F, :])
                b_row = small.tile([1, H], f32, tag="brow")
                nc.sync.dma_start(b_row[:], bias_in[layer, None, :])
                bt_ps = psum.tile([128, H], f32, tag="ps_m")
                nc.tensor.matmul(bt_ps[:], ones_row[:], b_row[:],
                                 start=True, stop=True)
                b_tile = small.tile([128, H], f32, tag="btile")
                nc.scalar.copy(b_tile[:], bt_ps[:])
                waug16 = small.tile([128, H + 2], f16, tag="waug16")
                nc.scalar.copy(waug16[:F, :], waug[:F, :])
                return waug16, b_tile

            xg_bufs = [const.tile([F_IN, 7 * 128], f16, tag=f"xg{i}",
                                  name=f"xg{i}") for i in range(2)]
            xg_state = {}

            def proj_block(layer, t, waug):
                """Project block t of `layer`, stage the packed 136B table
                rows, flush per group, and trigger the half-AllGathers."""
                F = F_IN if layer == 0 else H
                if layer == 0:
                    # batched x loads: one DMA per 8 blocks keeps the sync
                    # queue short so the staging flushes aren't delayed
                    if t % 7 == 0:
                        xg_state[0] = xg_bufs[(t // 7) % 2]
                        hi = min((t + 7) * 128, NPC)
                        nc.sync.dma_start(xg_state[0][:, 0:hi - t * 128],
                                          x_in[:, t * 128:hi])
                    xT_sb = xg_state[0][:, (t % 7) * 128:(t % 7 + 1) * 128]
                else:
                    xt = x_buf[:, t * H:(t + 1) * H]
                    xT_ps = psum.tile([H, 128], f16, tag="ps_t")
                    nc.tensor.transpose(xT_ps[:], xt, ident16[:])
                    xT_sb = work.tile([H, 128], f16, tag="xTsb")
                    nc.scalar.copy(xT_sb[:], xT_ps[:])
                h_ps = psum.tile([128, H + 2], f32, tag="ps_m")
                xT_ap = xT_sb if layer == 0 else xT_sb[:]
                nc.tensor.matmul(h_ps[:], xT_ap, waug[:F, :],
                                 start=True, stop=True)
                # group staging (partition-major compact rows)
                G = GA if t < BLOCKS_A else GB
                t0 = t if t < BLOCKS_A else t - BLOCKS_A
                if t0 % G == 0:
                    stage_state[layer] = work.tile([128, G * ELEM], f32,
                                                   tag="rowstg",
                                                   name="rowstg")
                stg = stage_state[layer]
                j = t0 % G
                stg16 = stg[:].bitcast(f16)
                nc.scalar.copy(stg16[:, j * 2 * ELEM:j * 2 * ELEM + H],
                               h_ps[:, 0:H])
                nc.scalar.activation(stg[:, j * ELEM + 32:j * ELEM + 33],
                                     h_ps[:, H:H + 1], AF.Identity,
                                     bias=amask[:, t:t + 1])
                nc.vector.tensor_copy(alphad[:, t:t + 1], h_ps[:, H + 1:H + 2])
                nc.vector.tensor_copy(alphas[:, t:t + 1], h_ps[:, H:H + 1])
                nc.vector.tensor_copy(h_buf[:, t * H:(t + 1) * H],
                                      h_ps[:, 0:H])
                if j == G - 1:
                    cls = "a" if t < BLOCKS_A else "b"
                    for c2, nm, lo, hi in TAB_SPLITS:
                        if c2 == cls and lo <= t0 - j and t0 < hi:
                            nc.sync.dma_start(
                                tab_own[nm][:, t0 - j - lo:t0 + 1 - lo, :],
                                stg[:].rearrange("p (g e) -> p g e", e=ELEM))


            def fill_split(split, eng=None):
                cls, nm, lo, hi = split
                tgt, nblk = ((sb_ta3, BLOCKS_A) if cls == "a"
                             else (sb_tb3, BLOCKS_B))
                for c in range(NC):
                    (eng or nc.sync).dma_start(
                        tgt[:, c * nblk + lo:c * nblk + hi, :],
                        tab_full[nm][c])

            def fills(skip_b2=False):
                for split in TAB_SPLITS:
                    if skip_b2 and split[1] == "b2":
                        continue
                    fill_split(split)

            offs_a = np.concatenate([[0], np.cumsum(128 * RL)]).astype(int)
            offs_b = np.concatenate([[0], np.cumsum(128 * RH)]).astype(int)

            # Per-class grid rings: A-class gathers run LOOKAHEAD blocks ahead
            # of the B-class + compute stream, so at a layer boundary the
            # queues hold W blocks of A-work while the B-half AllGather+fill
            # of the new layer completes.  Bump-allocated block offsets into
            # one const tile per class (same offsets every layer).
            def ring_offsets(sizes, window):
                cap = max(sum(sizes[m:m + window + 1])
                          for m in range(len(sizes))) + max(sizes)
                offs = []
                cur = 0
                for b, s in enumerate(sizes):
                    if cur + s > cap:
                        cur = 0
                    for j in range(max(0, b - window), b):
                        assert (cur + s <= offs[j]
                                or cur >= offs[j] + sizes[j]), (b, j)
                    offs.append(cur)
                    cur += s
                return offs, cap

            sizes_a = [int(RL[b]) * ELEM for b in range(BLOCKS)]
            sizes_b = [int(RH[b]) * ELEM for b in range(BLOCKS)]
            offA, CAP_A = ring_offsets(sizes_a, LOOKAHEAD + 2)
            offB, CAP_B = ring_offsets(sizes_b, 5)
            gridA_buf = const.tile([128, CAP_A], f32, tag="gridA")
            gridB_buf = const.tile([128, CAP_B], f32, tag="gridB")

            def grid_view(buf, off, rounds):
                return buf[:, off:off + rounds * ELEM].rearrange(
                    "p (r h) -> p r h", h=ELEM)

            def build_chunks(sizes, offs_ring, offs_idx):
                """Fuse each class's per-block gathers into CHUNK-slot
                instructions spanning consecutive blocks (their ring regions
                are bump-adjacent), splitting at ring wraps.  Returns
                {flush_block: [(idx_off, ring_off, n_slots), ...]}."""
                out = {}
                pend = []          # (block, idx_off, ring_off, n_slots)
                pn = 0

                def flush():
                    nonlocal pend, pn
                    if not pn:
                        return
                    # keyed by FIRST covered block: the chunk must be issued
                    # before that block's edge_compute
                    out.setdefault(pend[0][0], []).append(
                        (pend[0][1], pend[0][2], pn))
                    pend = []
                    pn = 0

                for b in range(BLOCKS):
                    n = sizes[b] // ELEM * 128
                    if pend and offs_ring[b] == 0:
                        flush()        # ring wrapped before this block
                    done = 0
                    while done < n:
                        take = min(CHUNK - pn, n - done)
                        pend.append((b, offs_idx[b] + done,
                                     offs_ring[b] + done // 128 * ELEM, take))
                        pn += take
                        done += take
                        if pn == CHUNK:
                            flush()
                    if b == BLOCKS - 1:
                        flush()
                return out

            chunksA = build_chunks(sizes_a, offA, [int(v) for v in offs_a])
            chunksB = build_chunks(sizes_b, offB, [int(v) for v in offs_b])

            def issue_chunks(chunks, b, buf, isb):
                for idx_off, ring_off, n in chunks.get(b, ()):
                    reg = None
                    if n == CHUNK:
                        if reg_full[0] is None:
                            reg_full[0] = nc.gpsimd.to_reg(CHUNK)
                        reg = reg_full[0]
                    _gather_sbuf(
                        nc,
                        buf[:, ring_off:ring_off + (n // 128) * ELEM]
                        .rearrange("p (r h) -> p r h", h=ELEM),
                        sb_tab_a[:] if buf is gridA_buf else sb_tab_b[:],
                        isb[:, idx_off // 16:(idx_off + n) // 16],
                        n, ELEM,
                        queue_num=self_q[0] % 4,
                        reg=reg,
                    )
                    self_q[0] += 1

            def edge_compute(layer, b, b_tile):
                rl, rh, rt = int(RL[b]), int(RH[b]), R_TOT[b]
                grA = grid_view(gridA_buf, offA[b], rl)
                grB = grid_view(gridB_buf, offB[b], rh)
                tbuf = work.tile([128, R_MAX], f32, tag="tbuf")
                nc.scalar.activation(tbuf[:, 0:rl], grA[:, 0:rl, 32],
                                     AF.Identity, bias=alphad[:, b:b + 1])
                nc.scalar.activation(tbuf[:, rl:rt], grB[:, 0:rh, 32],
                                     AF.Identity, bias=alphad[:, b:b + 1])
                nc.vector.scalar_tensor_tensor(
                    out=tbuf[:, 0:rt], in0=tbuf[:, 0:rt],
                    scalar=NEG_SLOPE, in1=tbuf[:, 0:rt],
                    op0=ALU.mult, op1=ALU.max)
                p_t = work.tile([128, R_MAX], f16, tag="ptile")
                den = small.tile([128, 1], f32, tag="den")
                nc.scalar.activation(p_t[:, 0:rt], tbuf[:, 0:rt], AF.Exp,
                                     bias=ebias[:, 0:1], accum_out=den[:])
                hA = (gridA_buf[:, offA[b]:offA[b] + rl * ELEM].bitcast(f16)
                      .rearrange("p (r h) -> p r h", h=2 * ELEM)[:, :, 0:H])
                hB = (gridB_buf[:, offB[b]:offB[b] + rh * ELEM].bitcast(f16)
                      .rearrange("p (r h) -> p r h", h=2 * ELEM)[:, :, 0:H])

                # weighted multiply + halving-tree reduction, in segments of
                # at most R_CAP rounds so the wt scratch stays small (only
                # blocks 0 and 25 exceed R_CAP); contiguous in-place f16
                # adds replace the old strided (transposed) reduce_sum
                num_t = work.tile([128, H], f32, tag="num")
                wt = work.tile([128, H * R_CAP], f16, tag="wtile")
                wt3 = wt[:].rearrange("p (r f) -> p r f", f=H)

                def seg_mult(r0, r1):
                    """wt3[0:r1-r0] = h rows (A/B concat) * p for rounds
                    [r0, r1)."""
                    n_a = max(0, min(rl, r1) - r0)
                    if n_a > 0:
                        nc.vector.tensor_tensor(
                            out=wt3[:, 0:n_a, :], in0=hA[:, r0:r0 + n_a, :],
                            in1=p_t[:, r0:r0 + n_a].unsqueeze(2)
                            .to_broadcast([128, n_a, H]), op=ALU.mult)
                    n_b = r1 - r0 - n_a
                    if n_b > 0:
                        b0 = max(0, r0 - rl)
                        nc.vector.tensor_tensor(
                            out=wt3[:, n_a:n_a + n_b, :],
                            in0=hB[:, b0:b0 + n_b, :],
                            in1=p_t[:, r0 + n_a:r1].unsqueeze(2)
                            .to_broadcast([128, n_b, H]), op=ALU.mult)

                nseg = (rt + R_CAP - 1) // R_CAP
                for s in range(nseg):
                    r0, r1 = s * R_CAP, min((s + 1) * R_CAP, rt)
                    seg_mult(r0, r1)
                    m = r1 - r0
                    while m > 2:
                        h2 = m // 2
                        nc.vector.tensor_tensor(
                            out=wt3[:, 0:h2, :], in0=wt3[:, 0:h2, :],
                            in1=wt3[:, m - h2:m, :], op=ALU.add)
                        m = h2 + (m & 1)
                    if s == 0:
                        nc.vector.tensor_tensor(
                            out=num_t[:], in0=wt3[:, 0, :], in1=wt3[:, 1, :],
                            op=ALU.add)
                    else:
                        nc.vector.tensor_tensor(
                            out=num_t[:], in0=num_t[:], in1=wt3[:, 0, :],
                            op=ALU.add)
                        nc.vector.tensor_tensor(
                            out=num_t[:], in0=num_t[:], in1=wt3[:, 1, :],
                            op=ALU.add)
                # self-loop handled locally: p_self = exp(lrelu(as+ad) - C)
                # (tbuf is free after the exp; reuse two of its columns)
                zs = tbuf[:, 0:1]
                ps_self = tbuf[:, 1:2]
                nc.vector.tensor_tensor(out=zs, in0=alphas[:, b:b + 1],
                                        in1=alphad[:, b:b + 1], op=ALU.add)
                nc.vector.scalar_tensor_tensor(
                    out=zs, in0=zs, scalar=NEG_SLOPE, in1=zs,
                    op0=ALU.mult, op1=ALU.max)
                nc.scalar.activation(ps_self, zs, AF.Exp,
                                     bias=ebias[:, 0:1])
                nc.vector.tensor_tensor(out=den[:], in0=den[:],
                                        in1=ps_self, op=ALU.add)
                nc.vector.scalar_tensor_tensor(
                    out=num_t[:], in0=h_buf[:, b * H:(b + 1) * H],
                    scalar=ps_self, in1=num_t[:],
                    op0=ALU.mult, op1=ALU.add)
                num = num_t[:]
                nc.vector.tensor_scalar_max(den[:], den[:], 1e-30)
                recip = small.tile([128, 1], f32, tag="recip")
                nc.vector.reciprocal(recip[:], den[:])
                jk = jk_buf[:, b * H:(b + 1) * H]
                if layer < L - 1:
                    xn = x_buf[:, b * H:(b + 1) * H]
                    nc.vector.scalar_tensor_tensor(
                        out=xn, in0=num, scalar=recip[:, 0:1],
                        in1=b_tile[:], op0=ALU.mult, op1=ALU.add)
                    nc.scalar.activation(xn, xn, AF.Relu)
                    if layer == 0:
                        nc.scalar.copy(jk, xn)
                    else:
                        nc.vector.tensor_tensor(out=jk, in0=jk, in1=xn,
                                                op=ALU.max)
                else:
                    xn = work.tile([128, H], f16, tag="xnlast",
                                   name="xnlast")[:]
                    nc.vector.scalar_tensor_tensor(
                        out=xn, in0=num, scalar=recip[:, 0:1],
                        in1=b_tile[:], op0=ALU.mult, op1=ALU.add)
                    nc.vector.scalar_tensor_tensor(
                        out=jk, in0=xn, scalar=0.0, in1=jk,
                        op0=ALU.max, op1=ALU.max)

            def y_proj(t, wout_sb, bo_tile):
                jt = jk_buf[:, t * H:(t + 1) * H]
                jT_ps = psum.tile([H, 128], f16, tag="ps_t")
                nc.tensor.transpose(jT_ps[:], jt, ident16[:])
                jT_sb = work.tile([H, 128], f16, tag="jTsb")
                nc.scalar.copy(jT_sb[:], jT_ps[:])
                y_ps = psum.tile([128, OUT], f32, tag="ps_m")
                nc.tensor.matmul(y_ps[:], jT_sb[:], wout_sb[:],
                                 start=True, stop=True)
                y_sb = work.tile([128, OUT], f32, tag="ysb")
                nc.vector.tensor_tensor(out=y_sb[:], in0=y_ps[:],
                                        in1=bo_tile[:], op=ALU.add)
                nc.sync.dma_start(out_t[t * 128:(t + 1) * 128, :], y_sb[:])

            def fire_ag(nm):
                nc.gpsimd.collective_compute(
                    "AllGather", ALU.bypass,
                    replica_groups=[list(range(NC))],
                    ins=[tab_own[nm].opt()], outs=[tab_full[nm].opt()])

            # proj step after which each split's staging has fully landed
            AG_AT = {9: "a1", 24: "a2", 36: "b1", 48: "b2"}

            # ---- layer 0 projection (x from DRAM) ----
            waug, b_tile = prep_weights(0)
            for t in range(BLOCKS):
                proj_block(0, t, waug)
                if t in AG_AT:
                    fire_ag(AG_AT[t])
            fills()

            # ---- layers ----
            for layer in range(L):
                if layer < L - 1:
                    waug_n, b_tile_n = prep_weights(layer + 1)
                else:
                    wout_f32 = const.tile([H, OUT], f32, tag="woutf32")
                    nc.sync.dma_start(wout_f32[:], wout_in[:])
                    wout_sb = const.tile([H, OUT], f16, tag="wout")
                    nc.scalar.copy(wout_sb[:], wout_f32[:])
                    bo_row = const.tile([1, OUT], f32, tag="borow")
                    nc.sync.dma_start(bo_row[:], bout_in[:])
                    bo_ps = psum.tile([128, OUT], f32, tag="ps_m")
                    nc.tensor.matmul(bo_ps[:], ones_row[:], bo_row[:],
                                     start=True, stop=True)
                    bo_tile = const.tile([128, OUT], f32, tag="botile")
                    nc.scalar.copy(bo_tile[:], bo_ps[:])

                for i in range(BLOCKS + LOOKAHEAD):
                    if i < BLOCKS:
                        issue_chunks(chunksA, i, gridA_buf, idxa_sb)
                    if i == 0 and layer > 0:
                        fill_split(("b", "b1", 0, 12))
                    if i == 6 and layer > 0:
                        # deferred b2 AllGather + fill of THIS layer's table:
                        # placed after a few A-gather issues so the new
                        # layer's A stream dispatches ahead of the trigger's
                        # wait (the fill must FOLLOW the trigger in program
                        # order to bind to this layer's AllGather)
                        fire_ag("b2")
                        fill_split(("b", "b2", 12, BLOCKS_B))
                    if i >= LOOKAHEAD:
                        b = i - LOOKAHEAD
                        issue_chunks(chunksB, b, gridB_buf, idxb_sb)
                        edge_compute(layer, b, b_tile)
                        # burst the next layer's projection every PGRP blocks
                        # to keep its PE->ACT round trips off the per-block
                        # chain while still firing the AllGathers mid-stream
                        grp = PGRP if layer < L - 1 else PGRP_Y
                        if b % grp == grp - 1 or b == BLOCKS - 1:
                            for t in range(b - b % grp, b + 1):
                                if layer < L - 1:
                                    proj_block(layer + 1, t, waug_n)
                                    if t in AG_AT and AG_AT[t] != "b2":
                                        fire_ag(AG_AT[t])
                                else:
                                    y_proj(t, wout_sb, bo_tile)
                        # next layer's a-fills early: the AGs are long done,
                        # and this keeps them off the SP queue tail where
                        # they'd sit behind the last staging flush
                        if b == 44 and layer < L - 1:
                            fill_split(("a", "a1", 0, 10))
                            fill_split(("a", "a2", 10, BLOCKS_A))
                if layer < L - 1:
                    waug, b_tile = waug_n, b_tile_n

    return nc


# ---------------------------------------------------------------------------
# Entry point
# ---------------------------------------------------------------------------

def kernel(x, edge_index, W1, W23, a_src, a_dst, b, Wout, bout):
    import concourse.bacc as bacc
    from concourse import bass_utils

    x = np.asarray(x, np.float32)
    edge_index = np.asarray(edge_index)
    nodes_of, idx_a, idx_b, RL, RH = _preprocess(edge_index.astype(np.int64))

    n_idx_a = len(idx_a[0])
    n_idx_b = len(idx_b[0])

    nc = bacc.Bacc("TRN2", target_bir_lowering=False, debug=False, num_devices=NC,
                   num_swdge_queues=4, dynamic_dma_scratch_size=24576)
    _build(nc, RL, RH, n_idx_a, n_idx_b)
    nc.compile()

    in_maps = []
    for c in range(NC):
        nodes = nodes_of[c]
        x_own = np.zeros((NPC, F_IN), np.float32)
        valid = np.nonzero(nodes >= 0)[0]
        x_own[valid] = x[nodes[valid]]
        in_maps.append({
            "xT_own": np.ascontiguousarray(x_own.T.astype(np.float16)),
            "w1": np.asarray(W1, np.float32),
            "w23": np.asarray(W23, np.float32),
            "asrc": np.asarray(a_src, np.float32),
            "adst": np.asarray(a_dst, np.float32),
            "bias": np.asarray(b, np.float32),
            "wout": np.asarray(Wout, np.float32),
            "bout": np.asarray(bout, np.float32).reshape(1, OUT),
            "idx_a": _wrap_idx(idx_a[c]),
            "idx_b": _wrap_idx(idx_b[c]),
            "alpha_mask": _alpha_mask(),
        })

    res = bass_utils.run_bass_kernel_spmd(nc, in_maps, core_ids=list(range(NC)))
    global _last_results
    _last_results = res
    out = np.zeros((N, OUT), np.float32)
    for c in range(NC):
        y = res.results[c]["y"]
        nodes = nodes_of[c]
        valid = np.nonzero(nodes >= 0)[0]
        out[nodes[valid]] = y[valid]
    return out



# revision 59
# speedup vs baseline: 1.1606x; 1.1606x over previous
"""GAT+JumpingKnowledge GNN kernel for 8 Trainium2 NeuronCores.

Sharding: nodes are assigned to cores round-robin by global in-degree rank
(6250/core; profiles match across cores so the SPMD round maxima stay
tight).  Each core, per layer:
  - projects its own nodes' features h = x @ [W | W@a_src | W@a_dst] (f16)
  - stages packed 132B table rows [64 x f16 h | f32 alpha_src] and
    AllGathers them in four rank-range splits (a1/a2/b1/b2) that fire as
    soon as their projections land; the b2 trigger+fill are deferred into
    the next layer's gather stream so the in-order GpSimd sequencer never
    stalls on them
  - gathers, per dst-node "slot grid" (nodes on partitions, incoming-edge
    rounds on the free dim), the src rows of its edges with a custom
    SBUF-source dma_gather.  Descriptor GENERATION on the Q7 is the
    bottleneck (~2.2ns/slot, serialized), so gathers are fused into
    1024-descriptor chunks that span blocks; the A-class stream leads the
    B-class+compute stream by LOOKAHEAD blocks through per-class SBUF
    rings so queues stay fed across layer boundaries
  - computes the edge softmax (no max subtraction; logit range ~[-7, 7])
    and the weighted aggregation with a DVE multiply + in-place f16
    halving-tree reduction (contiguous adds, R_CAP-round segments)
  - self-loops never touch the gather path: their contribution is computed
    locally from per-block alpha_src/alpha_dst and the kept h copy
Final JK-max + output projection happen on the owned nodes; the host
reassembles the full [50000, 40] output via the node assignment.
"""

import numpy as np

# --- problem constants (hardcoded per harness contract) ---
N = 50000
E = 1600000
F_IN = 128
H = 64
L = 3
OUT = 40
NEG_SLOPE = 0.2
NC = 8
NPC_REAL = N // NC          # 6250 real nodes per core
BLOCKS = 49                 # ceil(6250/128)
NPC = BLOCKS * 128          # 6272 padded nodes per core
BLOCKS_A = 25               # blocks in table half A (local rows [0, 3200))
ROWS_A = BLOCKS_A * 128     # 3200
ROWS_B = NPC - ROWS_A       # 3072
TAB_A = NC * ROWS_A         # 25600 rows in gathered half-A table
TAB_B = NC * ROWS_B         # 24576
PAD_A = ROWS_A - 1          # local pad row 3199 (half A dummy)
DUMMY_A = PAD_A             # core 0's pad row in A-table coords
DUMMY_B = 6251 - ROWS_A     # core 0's pad row 6251 in B-table coords
ELEM = 33                   # gathered element: 33 f32 = 132B (64 f16 h + f32 alpha)
SB_BLOCKS = 1               # blocks per superblock (gather granularity)
ALPHA_NEG = -1.0e30


# ---------------------------------------------------------------------------
# Host-side graph preprocessing
# ---------------------------------------------------------------------------

def _fill_grid(Rn, slot_p, rows_vals, dummy):
    """Grid [Rn, 128] in i=r*128+p order; node p's edges fill rounds 0..k-1."""
    grid = np.full((int(Rn), 128), dummy, np.int64)
    o = np.argsort(slot_p, kind="stable")
    ps = slot_p[o]
    rv = rows_vals[o]
    first = np.searchsorted(ps, np.arange(128), side="left")
    ranks = np.arange(len(ps)) - first[ps]
    grid[ranks, ps] = rv
    return grid.reshape(-1)


def _preprocess(edge_index):
    """Self-loops are handled locally on-device (never gathered).  Nodes are
    assigned to cores round-robin by global in-degree rank so every core's
    per-block degree profile matches (tight cross-core round maxima).  Within
    each core the top-3199 nodes are class A (table rows [0, 3199)), the rest
    class B; each class is sorted by (max(ka,kb), ka+kb) desc into its rows."""
    src = edge_index[0].astype(np.int64)
    dst = edge_index[1].astype(np.int64)

    deg = np.bincount(dst, minlength=N)
    order = np.argsort(-deg, kind="stable")
    core_of = np.empty(N, np.int64)
    core_of[order] = np.arange(N) % NC
    lrank = np.empty(N, np.int64)
    lrank[order] = np.arange(N) // NC
    is_a_node = lrank < PAD_A

    sA = is_a_node[src]
    ka_n = np.zeros(N, np.int64)
    np.add.at(ka_n, dst[sA], 1)
    kb_n = np.zeros(N, np.int64)
    np.add.at(kb_n, dst[~sA], 1)

    nodes_of = np.full((NC, NPC), -1, np.int64)   # row -> global node id
    row_of = np.full(N, -1, np.int64)             # node -> row in its core
    RL = np.zeros(BLOCKS, np.int64)
    RH = np.zeros(BLOCKS, np.int64)
    for c in range(NC):
        nodes = np.where(core_of == c)[0]
        for cls, row0 in ((True, 0), (False, ROWS_A)):
            ids = nodes[is_a_node[nodes] == cls]
            o = ids[np.lexsort((-(ka_n[ids] + kb_n[ids]),
                                -np.maximum(ka_n[ids], kb_n[ids])))]
            nodes_of[c, row0:row0 + len(o)] = o
            row_of[o] = row0 + np.arange(len(o))
        kar = np.where(nodes_of[c] >= 0, ka_n[np.maximum(nodes_of[c], 0)], 0)
        kbr = np.where(nodes_of[c] >= 0, kb_n[np.maximum(nodes_of[c], 0)], 0)
        RL = np.maximum(RL, kar.reshape(BLOCKS, 128).max(axis=1))
        RH = np.maximum(RH, kbr.reshape(BLOCKS, 128).max(axis=1))

    src_core = core_of[src]
    src_row = row_of[src]
    e_is_a = src_row < ROWS_A
    rows_a_all = src_core * ROWS_A + src_row
    rows_b_all = src_core * ROWS_B + (src_row - ROWS_A)
    slot_all = row_of[dst]

    idx_a_cores, idx_b_cores = [], []
    for c in range(NC):
        m = core_of[dst] == c
        slot_of = slot_all[m]
        is_a = e_is_a[m]
        rows_a = rows_a_all[m]
        rows_b = rows_b_all[m]
        la, lb = [], []
        for bidx in range(BLOCKS):
            base = bidx * 128
            in_blk = (slot_of >= base) & (slot_of < base + 128)
            sel = in_blk & is_a
            la.append(_fill_grid(RL[bidx], slot_of[sel] - base, rows_a[sel],
                                 DUMMY_A))
            sel = in_blk & ~is_a
            lb.append(_fill_grid(RH[bidx], slot_of[sel] - base, rows_b[sel],
                                 DUMMY_B))
        idx_a_cores.append(np.concatenate(la).astype(np.int16))
        idx_b_cores.append(np.concatenate(lb).astype(np.int16))

    return nodes_of, idx_a_cores, idx_b_cores, RL, RH


def _alpha_mask():
    """[128, BLOCKS] f32: -1e30 on pad rows (3199, 6251..6271), else 0."""
    mask = np.zeros((NPC,), np.float32)
    mask[PAD_A] = ALPHA_NEG
    mask[6251:] = ALPHA_NEG
    return np.ascontiguousarray(mask.reshape(BLOCKS, 128).T)


def _wrap_idx(flat):
    """[num] -> [128, num//16] wrapped (i%16, i//16), replicated to 128 parts."""
    num = len(flat)
    assert num % 16 == 0
    w = flat.reshape(num // 16, 16).T
    return np.ascontiguousarray(np.tile(w, (8, 1))).astype(np.int16)


# ---------------------------------------------------------------------------
# Device kernel builder
# ---------------------------------------------------------------------------

def _gather_sbuf(nc, out_ap, in_ap, idxs_ap, num_idxs, elem_size, queue_num,
                 reg=None):
    """Non-transpose dma_gather from an SBUF-resident table.

    Mirrors concourse.bass.BassGpSimd.dma_gather minus its "SBUF source
    implies transpose" restriction: the Q7 ucode's SBUF addressing branch
    (token = idx % 128 -> partition, rank = idx // 128 -> free-dim stripe)
    is independent of the transpose flag, and the non-transpose RX side
    writes the standard [128, num_idxs/128, elem] grid layout.
    """
    import concourse.mybir as mybir

    eng = nc.gpsimd
    elem_bytes = elem_size * mybir.dt.size(in_ap.dtype)
    return eng.add_instruction(
        mybir.InstDMAGatherAnt(
            name=eng.bass.get_next_instruction_name(),
            ins=[
                eng.lower_ap(in_ap),
                eng.lower_ap(idxs_ap),
                eng.lower_val_access(reg if reg is not None
                                     else eng.to_reg(num_idxs)),
            ],
            outs=[eng.lower_ap(out_ap)],
            transpose=False,
            num_idxs=num_idxs,
            elem_size=elem_size,
            stride_bytes_256=0,
            gen_mode=0,
            single_packet=True,
            queue_num=queue_num,
            sbuf_tokens_per_rank=128,
            sbuf_free_dim_per_rank=elem_bytes,
            sbuf_free_dim_pad_per_rank=0,
            sbuf_byte_offset=0,
        )
    )


def _build(nc, RL, RH, n_idx_a, n_idx_b):
    import contextlib

    import concourse.mybir as mybir
    import concourse.tile as tile
    from concourse import library_config
    from concourse.masks import make_identity

    f32 = mybir.dt.float32
    f16 = mybir.dt.float16
    AF = mybir.ActivationFunctionType
    ALU = mybir.AluOpType

    # --- I/O ---
    # x is pre-transposed on the host so layer-0 projection feeds the PE
    # stationary operand straight from DRAM (no per-block PE transpose).
    x_in = nc.dram_tensor("xT_own", [F_IN, NPC], f16, kind="ExternalInput").ap()
    w1_in = nc.dram_tensor("w1", [F_IN, H], f32, kind="ExternalInput").ap()
    w23_in = nc.dram_tensor("w23", [L - 1, H, H], f32, kind="ExternalInput").ap()
    asrc_in = nc.dram_tensor("asrc", [L, H], f32, kind="ExternalInput").ap()
    adst_in = nc.dram_tensor("adst", [L, H], f32, kind="ExternalInput").ap()
    bias_in = nc.dram_tensor("bias", [L, H], f32, kind="ExternalInput").ap()
    wout_in = nc.dram_tensor("wout", [H, OUT], f32, kind="ExternalInput").ap()
    bout_in = nc.dram_tensor("bout", [1, OUT], f32, kind="ExternalInput").ap()
    idxa_in = nc.dram_tensor("idx_a", [128, n_idx_a // 16], mybir.dt.int16,
                             kind="ExternalInput").ap()
    idxb_in = nc.dram_tensor("idx_b", [128, n_idx_b // 16], mybir.dt.int16,
                             kind="ExternalInput").ap()
    amask_in = nc.dram_tensor("alpha_mask", [128, BLOCKS], f32,
                              kind="ExternalInput").ap()
    out_t = nc.dram_tensor("y", [NPC, OUT], f32, kind="ExternalOutput").ap()

    # --- internal DRAM ---
    # Compact partition-major tables: core-local row r lives at
    # [r % 128, r // 128, :], so the post-AllGather DRAM->SBUF fill runs at
    # line rate (one big descriptor per (core, partition)).
    BLOCKS_B = BLOCKS - BLOCKS_A
    # The table halves are AllGathered in four rank-range splits so each
    # collective fires as soon as its projections land; only the last split's
    # (short) chain remains exposed at a layer boundary.
    # (class, name, rank lo, rank hi) -- ranks are within the class table.
    TAB_SPLITS = [("a", "a1", 0, 10), ("a", "a2", 10, BLOCKS_A),
                  ("b", "b1", 0, 12), ("b", "b2", 12, BLOCKS_B)]
    tab_own = {}
    tab_full = {}
    for cls, nm, lo, hi in TAB_SPLITS:
        tab_own[nm] = nc.dram_tensor(f"tab_own_{nm}", [128, hi - lo, ELEM],
                                     f32, kind="Internal").ap()
        tab_full[nm] = nc.dram_tensor(f"tab_full_{nm}",
                                      [NC, 128, hi - lo, ELEM], f32,
                                      kind="Internal",
                                      addr_space="Shared").ap()

    R_TOT = [int(RL[b] + RH[b]) for b in range(BLOCKS)]
    R_MAX = max(R_TOT)
    CHUNK = 1024         # gather chunk (descriptors per SWDGE instruction)
    LOOKAHEAD = 10       # blocks the A-class gather stream leads by
    R_CAP = 38           # max rounds per multiply+tree segment (wt scratch)
    GA, GB = 5, 6        # row-store group sizes (25 = 5*5, 24 = 4*6)
    PGRP = 8             # next-layer projection burst size
    PGRP_Y = 8           # last-layer output projection burst size

    # Per-block queue: all of a block's gather chunks share one queue so a
    # blocked chunk (grid-pool WAR, pending fill) parks only its own queue.
    # Greedy-balance block costs (~R_TOT) across the 4 SWDGE queues.
    qload = [0] * 4
    QMAP = [0] * BLOCKS
    for b in sorted(range(BLOCKS), key=lambda x: -R_TOT[x]):
        q = min(range(4), key=lambda i: qload[i])
        QMAP[b] = q
        qload[q] += R_TOT[b]

    warm_in = nc.dram_tensor("cc_warm_in", [1, 16], f32, kind="Internal").ap()
    warm_out = nc.dram_tensor("cc_warm_out", [NC, 1, 16], f32, kind="Internal",
                              addr_space="Shared").ap()

    with tile.TileContext(nc) as tc:
        nc.gpsimd.load_library(library_config.mlp)

        with contextlib.ExitStack() as ctx:
            const = ctx.enter_context(tc.tile_pool(name="const", bufs=1))
            psum = ctx.enter_context(tc.tile_pool(name="psum", bufs=4, space="PSUM"))
            work = ctx.enter_context(tc.tile_pool(name="work", bufs=3))
            small = ctx.enter_context(tc.tile_pool(name="small", bufs=4))

            nc.gpsimd.collective_compute(
                "AllGather", ALU.bypass, replica_groups=[list(range(NC))],
                ins=[warm_in.opt()], outs=[warm_out.opt()])
            ident = const.tile([128, 128], f32, tag="ident")
            make_identity(nc, ident[:])
            ident16 = const.tile([128, 128], f16, tag="ident16")
            make_identity(nc, ident16[:])
            ones_row = const.tile([1, 128], f32, tag="ones")
            nc.vector.memset(ones_row[:], 1.0)
            idxa_sb = const.tile([128, n_idx_a // 16], mybir.dt.int16, tag="idxa")
            nc.sync.dma_start(idxa_sb[:], idxa_in[:])
            idxb_sb = const.tile([128, n_idx_b // 16], mybir.dt.int16, tag="idxb")
            nc.sync.dma_start(idxb_sb[:], idxb_in[:])
            x_buf = const.tile([128, BLOCKS * H], f16, tag="xbuf")
            jk_buf = const.tile([128, BLOCKS * H], f16, tag="jkbuf")
            sb_tab_a = const.tile([128, NC * BLOCKS_A * ELEM], f32, tag="taba")
            sb_tab_b = const.tile([128, NC * BLOCKS_B * ELEM], f32, tag="tabb")
            sb_ta3 = sb_tab_a[:].rearrange("p (k e) -> p k e", e=ELEM)
            sb_tb3 = sb_tab_b[:].rearrange("p (k e) -> p k e", e=ELEM)
            alphad = const.tile([128, BLOCKS], f32, tag="alphad")
            alphas = const.tile([128, BLOCKS], f32, tag="alphas")
            h_buf = const.tile([128, BLOCKS * H], f16, tag="hbuf")
            amask = const.tile([128, BLOCKS], f32, tag="amask")
            nc.sync.dma_start(amask[:], amask_in[:])
            ebias = const.tile([128, 1], f32, tag="ebias")
            nc.vector.memset(ebias[:], -2.772588722239781)

            self_q = [0]
            stage_state = {}

            def prep_weights(layer):
                """[W | W@a_src | W@a_dst] + bias broadcast tile for layer."""
                F = F_IN if layer == 0 else H
                w_ap = w1_in if layer == 0 else w23_in[layer - 1]
                waug = small.tile([128, H + 2], f32, tag="waug")
                nc.sync.dma_start(waug[:F, 0:H], w_ap)
                wt_ps = psum.tile([H, 128], f32, tag="ps_t")
                nc.tensor.transpose(wt_ps[:, :F], waug[:F, 0:H], ident[:F, :F])
                wt_sb = small.tile([H, 128], f32, tag="wtsb")
                nc.scalar.copy(wt_sb[:, :F], wt_ps[:, :F])
                a_cols = small.tile([H, 2], f32, tag="acols")
                nc.sync.dma_start(a_cols[:, 0:1], asrc_in[layer, :, None])
                nc.sync.dma_start(a_cols[:, 1:2], adst_in[layer, :, None])
                va_ps = psum.tile([128, 2], f32, tag="ps_m")
                nc.tensor.matmul(va_ps[:F, :], wt_sb[:, :F], a_cols[:],
                                 start=True, stop=True)
                nc.vector.tensor_copy(waug[:F, H:H + 2], va_ps[:F, :])
                b_row = small.tile([1, H], f32, tag="brow")
                nc.sync.dma_start(b_row[:], bias_in[layer, None, :])
                bt_ps = psum.tile([128, H], f32, tag="ps_m")
                nc.tensor.matmul(bt_ps[:], ones_row[:], b_row[:],
                                 start=True, stop=True)
                b_tile = small.tile([128, H], f32, tag="btile")
                nc.scalar.copy(b_tile[:], bt_ps[:])
                waug16 = small.tile([128, H + 2], f16, tag="waug16")
                nc.scalar.copy(waug16[:F, :], waug[:F, :])
                return waug16, b_tile

            xg_bufs = [const.tile([F_IN, 7 * 128], f16, tag=f"xg{i}",
                                  name=f"xg{i}") for i in range(2)]
            xg_state = {}

            def proj_block(layer, t, waug):
                """Project block t of `layer`, stage the packed 136B table
                rows, flush per group, and trigger the half-AllGathers."""
                F = F_IN if layer == 0 else H
                if layer == 0:
                    # batched x loads: one DMA per 8 blocks keeps the sync
                    # queue short so the staging flushes aren't delayed
                    if t % 7 == 0:
                        xg_state[0] = xg_bufs[(t // 7) % 2]
                        hi = min((t + 7) * 128, NPC)
                        nc.sync.dma_start(xg_state[0][:, 0:hi - t * 128],
                                          x_in[:, t * 128:hi])
                    xT_sb = xg_state[0][:, (t % 7) * 128:(t % 7 + 1) * 128]
                else:
                    xt = x_buf[:, t * H:(t + 1) * H]
                    xT_ps = psum.tile([H, 128], f16, tag="ps_t")
                    nc.tensor.transpose(xT_ps[:], xt, ident16[:])
                    xT_sb = work.tile([H, 128], f16, tag="xTsb")
                    nc.scalar.copy(xT_sb[:], xT_ps[:])
                h_ps = psum.tile([128, H + 2], f32, tag="ps_m")
                xT_ap = xT_sb if layer == 0 else xT_sb[:]
                nc.tensor.matmul(h_ps[:], xT_ap, waug[:F, :],
                                 start=True, stop=True)
                # group staging (partition-major compact rows)
                G = GA if t < BLOCKS_A else GB
                t0 = t if t < BLOCKS_A else t - BLOCKS_A
                if t0 % G == 0:
                    stage_state[layer] = work.tile([128, G * ELEM], f32,
                                                   tag="rowstg",
                                                   name="rowstg")
                stg = stage_state[layer]
                j = t0 % G
                stg16 = stg[:].bitcast(f16)
                nc.scalar.copy(stg16[:, j * 2 * ELEM:j * 2 * ELEM + H],
                               h_ps[:, 0:H])
                nc.scalar.activation(stg[:, j * ELEM + 32:j * ELEM + 33],
                                     h_ps[:, H:H + 1], AF.Identity,
                                     bias=amask[:, t:t + 1])
                nc.vector.tensor_copy(alphad[:, t:t + 1], h_ps[:, H + 1:H + 2])
                nc.vector.tensor_copy(alphas[:, t:t + 1], h_ps[:, H:H + 1])
                nc.vector.tensor_copy(h_buf[:, t * H:(t + 1) * H],
                                      h_ps[:, 0:H])
                if j == G - 1:
                    cls = "a" if t < BLOCKS_A else "b"
                    for c2, nm, lo, hi in TAB_SPLITS:
                        if c2 == cls and lo <= t0 - j and t0 < hi:
                            nc.sync.dma_start(
                                tab_own[nm][:, t0 - j - lo:t0 + 1 - lo, :],
                                stg[:].rearrange("p (g e) -> p g e", e=ELEM))


            def fill_split(split, eng=None):
                cls, nm, lo, hi = split
                tgt, nblk = ((sb_ta3, BLOCKS_A) if cls == "a"
                             else (sb_tb3, BLOCKS_B))
                for c in range(NC):
                    (eng or nc.sync).dma_start(
                        tgt[:, c * nblk + lo:c * nblk + hi, :],
                        tab_full[nm][c])

            def fills(skip_b2=False):
                for split in TAB_SPLITS:
                    if skip_b2 and split[1] == "b2":
                        continue
                    fill_split(split)

            offs_a = np.concatenate([[0], np.cumsum(128 * RL)]).astype(int)
            offs_b = np.concatenate([[0], np.cumsum(128 * RH)]).astype(int)

            # Per-class grid rings: A-class gathers run LOOKAHEAD blocks ahead
            # of the B-class + compute stream, so at a layer boundary the
            # queues hold W blocks of A-work while the B-half AllGather+fill
            # of the new layer completes.  Bump-allocated block offsets into
            # one const tile per class (same offsets every layer).
            def ring_offsets(sizes, window):
                cap = max(sum(sizes[m:m + window + 1])
                          for m in range(len(sizes))) + max(sizes)
                offs = []
                cur = 0
                for b, s in enumerate(sizes):
                    if cur + s > cap:
                        cur = 0
                    for j in range(max(0, b - window), b):
                        assert (cur + s <= offs[j]
                                or cur >= offs[j] + sizes[j]), (b, j)
                    offs.append(cur)
                    cur += s
                return offs, cap

            sizes_a = [int(RL[b]) * ELEM for b in range(BLOCKS)]
            sizes_b = [int(RH[b]) * ELEM for b in range(BLOCKS)]
            offA, CAP_A = ring_offsets(sizes_a, LOOKAHEAD + 2)
            offB, CAP_B = ring_offsets(sizes_b, 5)
            gridA_buf = const.tile([128, CAP_A], f32, tag="gridA")
            gridB_buf = const.tile([128, CAP_B], f32, tag="gridB")

            def grid_view(buf, off, rounds):
                return buf[:, off:off + rounds * ELEM].rearrange(
                    "p (r h) -> p r h", h=ELEM)

            def build_chunks(sizes, offs_ring, offs_idx):
                """Fuse each class's per-block gathers into CHUNK-slot
                instructions spanning consecutive blocks (their ring regions
                are bump-adjacent), splitting at ring wraps.  Returns
                {flush_block: [(idx_off, ring_off, n_slots), ...]}."""
                out = {}
                pend = []          # (block, idx_off, ring_off, n_slots)
                pn = 0

                def flush():
                    nonlocal pend, pn
                    if not pn:
                        return
                    # keyed by FIRST covered block: the chunk must be issued
                    # before that block's edge_compute
                    out.setdefault(pend[0][0], []).append(
                        (pend[0][1], pend[0][2], pn))
                    pend = []
                    pn = 0

                for b in range(BLOCKS):
                    n = sizes[b] // ELEM * 128
                    if pend and offs_ring[b] == 0:
                        flush()        # ring wrapped before this block
                    done = 0
                    while done < n:
                        take = min(CHUNK - pn, n - done)
                        pend.append((b, offs_idx[b] + done,
                                     offs_ring[b] + done // 128 * ELEM, take))
                        pn += take
                        done += take
                        if pn == CHUNK:
                            flush()
                    if b == BLOCKS - 1:
                        flush()
                return out

            chunksA = build_chunks(sizes_a, offA, [int(v) for v in offs_a])
            chunksB = build_chunks(sizes_b, offB, [int(v) for v in offs_b])

            def issue_chunks(chunks, b, buf, isb):
                for idx_off, ring_off, n in chunks.get(b, ()):
                    _gather_sbuf(
                        nc,
                        buf[:, ring_off:ring_off + (n // 128) * ELEM]
                        .rearrange("p (r h) -> p r h", h=ELEM),
                        sb_tab_a[:] if buf is gridA_buf else sb_tab_b[:],
                        isb[:, idx_off // 16:(idx_off + n) // 16],
                        n, ELEM,
                        queue_num=self_q[0] % 4,
                    )
                    self_q[0] += 1

            def edge_compute(layer, b, b_tile):
                rl, rh, rt = int(RL[b]), int(RH[b]), R_TOT[b]
                grA = grid_view(gridA_buf, offA[b], rl)
                grB = grid_view(gridB_buf, offB[b], rh)
                tbuf = work.tile([128, R_MAX], f32, tag="tbuf")
                nc.scalar.activation(tbuf[:, 0:rl], grA[:, 0:rl, 32],
                                     AF.Identity, bias=alphad[:, b:b + 1])
                nc.scalar.activation(tbuf[:, rl:rt], grB[:, 0:rh, 32],
                                     AF.Identity, bias=alphad[:, b:b + 1])
                nc.vector.scalar_tensor_tensor(
                    out=tbuf[:, 0:rt], in0=tbuf[:, 0:rt],
                    scalar=NEG_SLOPE, in1=tbuf[:, 0:rt],
                    op0=ALU.mult, op1=ALU.max)
                p_t = work.tile([128, R_MAX], f16, tag="ptile")
                den = small.tile([128, 1], f32, tag="den")
                nc.scalar.activation(p_t[:, 0:rt], tbuf[:, 0:rt], AF.Exp,
                                     bias=ebias[:, 0:1], accum_out=den[:])
                hA = (gridA_buf[:, offA[b]:offA[b] + rl * ELEM].bitcast(f16)
                      .rearrange("p (r h) -> p r h", h=2 * ELEM)[:, :, 0:H])
                hB = (gridB_buf[:, offB[b]:offB[b] + rh * ELEM].bitcast(f16)
                      .rearrange("p (r h) -> p r h", h=2 * ELEM)[:, :, 0:H])

                # weighted multiply + halving-tree reduction, in segments of
                # at most R_CAP rounds so the wt scratch stays small (only
                # blocks 0 and 25 exceed R_CAP); contiguous in-place f16
                # adds replace the old strided (transposed) reduce_sum
                num_t = work.tile([128, H], f32, tag="num")
                wt = work.tile([128, H * R_CAP], f16, tag="wtile")
                wt3 = wt[:].rearrange("p (r f) -> p r f", f=H)

                def seg_mult(r0, r1):
                    """wt3[0:r1-r0] = h rows (A/B concat) * p for rounds
                    [r0, r1)."""
                    n_a = max(0, min(rl, r1) - r0)
                    if n_a > 0:
                        nc.vector.tensor_tensor(
                            out=wt3[:, 0:n_a, :], in0=hA[:, r0:r0 + n_a, :],
                            in1=p_t[:, r0:r0 + n_a].unsqueeze(2)
                            .to_broadcast([128, n_a, H]), op=ALU.mult)
                    n_b = r1 - r0 - n_a
                    if n_b > 0:
                        b0 = max(0, r0 - rl)
                        nc.vector.tensor_tensor(
                            out=wt3[:, n_a:n_a + n_b, :],
                            in0=hB[:, b0:b0 + n_b, :],
                            in1=p_t[:, r0 + n_a:r1].unsqueeze(2)
                            .to_broadcast([128, n_b, H]), op=ALU.mult)

                nseg = (rt + R_CAP - 1) // R_CAP
                for s in range(nseg):
                    r0, r1 = s * R_CAP, min((s + 1) * R_CAP, rt)
                    seg_mult(r0, r1)
                    m = r1 - r0
                    while m > 2:
                        h2 = m // 2
                        nc.vector.tensor_tensor(
                            out=wt3[:, 0:h2, :], in0=wt3[:, 0:h2, :],
                            in1=wt3[:, m - h2:m, :], op=ALU.add)
                        m = h2 + (m & 1)
                    if s == 0:
                        nc.vector.tensor_tensor(
                            out=num_t[:], in0=wt3[:, 0, :], in1=wt3[:, 1, :],
                            op=ALU.add)
                    else:
                        nc.vector.tensor_tensor(
                            out=num_t[:], in0=num_t[:], in1=wt3[:, 0, :],
                            op=ALU.add)
                        nc.vector.tensor_tensor(
                            out=num_t[:], in0=num_t[:], in1=wt3[:, 1, :],
                            op=ALU.add)
                # self-loop handled locally: p_self = exp(lrelu(as+ad) - C)
                # (tbuf is free after the exp; reuse two of its columns)
                zs = tbuf[:, 0:1]
                ps_self = tbuf[:, 1:2]
                nc.vector.tensor_tensor(out=zs, in0=alphas[:, b:b + 1],
                                        in1=alphad[:, b:b + 1], op=ALU.add)
                nc.vector.scalar_tensor_tensor(
                    out=zs, in0=zs, scalar=NEG_SLOPE, in1=zs,
                    op0=ALU.mult, op1=ALU.max)
                nc.scalar.activation(ps_self, zs, AF.Exp,
                                     bias=ebias[:, 0:1])
                nc.vector.tensor_tensor(out=den[:], in0=den[:],
                                        in1=ps_self, op=ALU.add)
                nc.vector.scalar_tensor_tensor(
                    out=num_t[:], in0=h_buf[:, b * H:(b + 1) * H],
                    scalar=ps_self, in1=num_t[:],
                    op0=ALU.mult, op1=ALU.add)
                num = num_t[:]
                nc.vector.tensor_scalar_max(den[:], den[:], 1e-30)
                recip = small.tile([128, 1], f32, tag="recip")
                nc.vector.reciprocal(recip[:], den[:])
                jk = jk_buf[:, b * H:(b + 1) * H]
                if layer < L - 1:
                    xn = x_buf[:, b * H:(b + 1) * H]
                    nc.vector.scalar_tensor_tensor(
                        out=xn, in0=num, scalar=recip[:, 0:1],
                        in1=b_tile[:], op0=ALU.mult, op1=ALU.add)
                    nc.scalar.activation(xn, xn, AF.Relu)
                    if layer == 0:
                        nc.scalar.copy(jk, xn)
                    else:
                        nc.vector.tensor_tensor(out=jk, in0=jk, in1=xn,
                                                op=ALU.max)
                else:
                    xn = work.tile([128, H], f16, tag="xnlast",
                                   name="xnlast")[:]
                    nc.vector.scalar_tensor_tensor(
                        out=xn, in0=num, scalar=recip[:, 0:1],
                        in1=b_tile[:], op0=ALU.mult, op1=ALU.add)
                    nc.vector.scalar_tensor_tensor(
                        out=jk, in0=xn, scalar=0.0, in1=jk,
                        op0=ALU.max, op1=ALU.max)

            def y_proj(t, wout_sb, bo_tile):
                jt = jk_buf[:, t * H:(t + 1) * H]
                jT_ps = psum.tile([H, 128], f16, tag="ps_t")
                nc.tensor.transpose(jT_ps[:], jt, ident16[:])
                jT_sb = work.tile([H, 128], f16, tag="jTsb")
                nc.scalar.copy(jT_sb[:], jT_ps[:])
                y_ps = psum.tile([128, OUT], f32, tag="ps_m")
                nc.tensor.matmul(y_ps[:], jT_sb[:], wout_sb[:],
                                 start=True, stop=True)
                y_sb = work.tile([128, OUT], f32, tag="ysb")
                nc.vector.tensor_tensor(out=y_sb[:], in0=y_ps[:],
                                        in1=bo_tile[:], op=ALU.add)
                nc.sync.dma_start(out_t[t * 128:(t + 1) * 128, :], y_sb[:])

            def fire_ag(nm):
                nc.gpsimd.collective_compute(
                    "AllGather", ALU.bypass,
                    replica_groups=[list(range(NC))],
                    ins=[tab_own[nm].opt()], outs=[tab_full[nm].opt()])

            # proj step after which each split's staging has fully landed
            AG_AT = {9: "a1", 24: "a2", 36: "b1", 48: "b2"}

            # ---- layer 0 projection (x from DRAM) ----
            waug, b_tile = prep_weights(0)
            for t in range(BLOCKS):
                proj_block(0, t, waug)
                if t in AG_AT:
                    fire_ag(AG_AT[t])
            fills()

            # ---- layers ----
            for layer in range(L):
                if layer < L - 1:
                    waug_n, b_tile_n = prep_weights(layer + 1)
                else:
                    wout_f32 = const.tile([H, OUT], f32, tag="woutf32")
                    nc.sync.dma_start(wout_f32[:], wout_in[:])
                    wout_sb = const.tile([H, OUT], f16, tag="wout")
                    nc.scalar.copy(wout_sb[:], wout_f32[:])
                    bo_row = const.tile([1, OUT], f32, tag="borow")
                    nc.sync.dma_start(bo_row[:], bout_in[:])
                    bo_ps = psum.tile([128, OUT], f32, tag="ps_m")
                    nc.tensor.matmul(bo_ps[:], ones_row[:], bo_row[:],
                                     start=True, stop=True)
                    bo_tile = const.tile([128, OUT], f32, tag="botile")
                    nc.scalar.copy(bo_tile[:], bo_ps[:])

                for i in range(BLOCKS + LOOKAHEAD):
                    if i < BLOCKS:
                        issue_chunks(chunksA, i, gridA_buf, idxa_sb)
                    if i == 0 and layer > 0:
                        fill_split(("b", "b1", 0, 12))
                    if i == 6 and layer > 0:
                        # deferred b2 AllGather + fill of THIS layer's table:
                        # placed after a few A-gather issues so the new
                        # layer's A stream dispatches ahead of the trigger's
                        # wait (the fill must FOLLOW the trigger in program
                        # order to bind to this layer's AllGather)
                        fire_ag("b2")
                        fill_split(("b", "b2", 12, BLOCKS_B))
                    if i >= LOOKAHEAD:
                        b = i - LOOKAHEAD
                        issue_chunks(chunksB, b, gridB_buf, idxb_sb)
                        edge_compute(layer, b, b_tile)
                        # burst the next layer's projection every PGRP blocks
                        # to keep its PE->ACT round trips off the per-block
                        # chain while still firing the AllGathers mid-stream
                        grp = PGRP if layer < L - 1 else PGRP_Y
                        if b % grp == grp - 1 or b == BLOCKS - 1:
                            for t in range(b - b % grp, b + 1):
                                if layer < L - 1:
                                    proj_block(layer + 1, t, waug_n)
                                    if t in AG_AT and AG_AT[t] != "b2":
                                        fire_ag(AG_AT[t])
                                else:
                                    y_proj(t, wout_sb, bo_tile)
                        # next layer's a-fills early: the AGs are long done,
                        # and this keeps them off the SP queue tail where
                        # they'd sit behind the last staging flush
                        if b == 44 and layer < L - 1:
                            fill_split(("a", "a1", 0, 10))
                            fill_split(("a", "a2", 10, BLOCKS_A))
                if layer < L - 1:
                    waug, b_tile = waug_n, b_tile_n

    return nc


# ---------------------------------------------------------------------------
# Entry point
# ---------------------------------------------------------------------------

def kernel(x, edge_index, W1, W23, a_src, a_dst, b, Wout, bout):
    import concourse.bacc as bacc
    from concourse import bass_utils

    x = np.asarray(x, np.float32)
    edge_index = np.asarray(edge_index)
    nodes_of, idx_a, idx_b, RL, RH = _preprocess(edge_index.astype(np.int64))

    n_idx_a = len(idx_a[0])
    n_idx_b = len(idx_b[0])

    nc = bacc.Bacc("TRN2", target_bir_lowering=False, debug=False, num_devices=NC,
                   num_swdge_queues=4, dynamic_dma_scratch_size=24576)
    _build(nc, RL, RH, n_idx_a, n_idx_b)
    nc.compile()

    in_maps = []
    for c in range(NC):
        nodes = nodes_of[c]
        x_own = np.zeros((NPC, F_IN), np.float32)
        valid = np.nonzero(nodes >= 0)[0]
        x_own[valid] = x[nodes[valid]]
        in_maps.append({
            "xT_own": np.ascontiguousarray(x_own.T.astype(np.float16)),
            "w1": np.asarray(W1, np.float32),
            "w23": np.asarray(W23, np.float32),
            "asrc": np.asarray(a_src, np.float32),
            "adst": np.asarray(a_dst, np.float32),
            "bias": np.asarray(b, np.float32),
            "wout": np.asarray(Wout, np.float32),
            "bout": np.asarray(bout, np.float32).reshape(1, OUT),
            "idx_a": _wrap_idx(idx_a[c]),
            "idx_b": _wrap_idx(idx_b[c]),
            "alpha_mask": _alpha_mask(),
        })

    res = bass_utils.run_bass_kernel_spmd(nc, in_maps, core_ids=list(range(NC)))
    global _last_results
    _last_results = res
    out = np.zeros((N, OUT), np.float32)
    for c in range(NC):
        y = res.results[c]["y"]
        nodes = nodes_of[c]
        valid = np.nonzero(nodes >= 0)[0]
        out[nodes[valid]] = y[valid]
    return out



# revision 61
# speedup vs baseline: 1.1689x; 1.0071x over previous
"""GAT+JumpingKnowledge GNN kernel for 8 Trainium2 NeuronCores.

Sharding: nodes are assigned to cores round-robin by global in-degree rank
(6250/core; profiles match across cores so the SPMD round maxima stay
tight).  Each core, per layer:
  - projects its own nodes' features h = x @ [W | W@a_src | W@a_dst] (f16)
  - stages packed 132B table rows [64 x f16 h | f32 alpha_src] and
    AllGathers them in four rank-range splits (a1/a2/b1/b2) that fire as
    soon as their projections land; the b2 trigger+fill are deferred into
    the next layer's gather stream so the in-order GpSimd sequencer never
    stalls on them
  - gathers, per dst-node "slot grid" (nodes on partitions, incoming-edge
    rounds on the free dim), the src rows of its edges with a custom
    SBUF-source dma_gather.  Descriptor GENERATION on the Q7 is the
    bottleneck (~2.2ns/slot, serialized), so gathers are fused into
    1024-descriptor chunks that span blocks; the A-class stream leads the
    B-class+compute stream by LOOKAHEAD blocks through per-class SBUF
    rings so queues stay fed across layer boundaries
  - computes the edge softmax (no max subtraction; logit range ~[-7, 7])
    and the weighted aggregation with a DVE multiply + in-place f16
    halving-tree reduction (contiguous adds, R_CAP-round segments)
  - self-loops never touch the gather path: their contribution is computed
    locally from per-block alpha_src/alpha_dst and the kept h copy
Final JK-max + output projection happen on the owned nodes; the host
reassembles the full [50000, 40] output via the node assignment.
"""

import numpy as np

# --- problem constants (hardcoded per harness contract) ---
N = 50000
E = 1600000
F_IN = 128
H = 64
L = 3
OUT = 40
NEG_SLOPE = 0.2
NC = 8
NPC_REAL = N // NC          # 6250 real nodes per core
BLOCKS = 49                 # ceil(6250/128)
NPC = BLOCKS * 128          # 6272 padded nodes per core
BLOCKS_A = 25               # blocks in table half A (local rows [0, 3200))
ROWS_A = BLOCKS_A * 128     # 3200
ROWS_B = NPC - ROWS_A       # 3072
TAB_A = NC * ROWS_A         # 25600 rows in gathered half-A table
TAB_B = NC * ROWS_B         # 24576
PAD_A = ROWS_A - 1          # local pad row 3199 (half A dummy)
DUMMY_A = PAD_A             # core 0's pad row in A-table coords
DUMMY_B = 6251 - ROWS_A     # core 0's pad row 6251 in B-table coords
ELEM = 33                   # gathered element: 33 f32 = 132B (64 f16 h + f32 alpha)
SB_BLOCKS = 1               # blocks per superblock (gather granularity)
ALPHA_NEG = -1.0e30


# ---------------------------------------------------------------------------
# Host-side graph preprocessing
# ---------------------------------------------------------------------------

def _fill_grid(Rn, slot_p, rows_vals, dummy):
    """Grid [Rn, 128] in i=r*128+p order; node p's edges fill rounds 0..k-1."""
    grid = np.full((int(Rn), 128), dummy, np.int64)
    o = np.argsort(slot_p, kind="stable")
    ps = slot_p[o]
    rv = rows_vals[o]
    first = np.searchsorted(ps, np.arange(128), side="left")
    ranks = np.arange(len(ps)) - first[ps]
    grid[ranks, ps] = rv
    return grid.reshape(-1)


def _preprocess(edge_index):
    """Self-loops are handled locally on-device (never gathered).  Nodes are
    assigned to cores round-robin by global in-degree rank so every core's
    per-block degree profile matches (tight cross-core round maxima).  Within
    each core the top-3199 nodes are class A (table rows [0, 3199)), the rest
    class B; each class is sorted by (max(ka,kb), ka+kb) desc into its rows."""
    src = edge_index[0].astype(np.int64)
    dst = edge_index[1].astype(np.int64)

    deg = np.bincount(dst, minlength=N)
    order = np.argsort(-deg, kind="stable")
    core_of = np.empty(N, np.int64)
    core_of[order] = np.arange(N) % NC
    lrank = np.empty(N, np.int64)
    lrank[order] = np.arange(N) // NC
    is_a_node = lrank < PAD_A

    sA = is_a_node[src]
    ka_n = np.zeros(N, np.int64)
    np.add.at(ka_n, dst[sA], 1)
    kb_n = np.zeros(N, np.int64)
    np.add.at(kb_n, dst[~sA], 1)

    nodes_of = np.full((NC, NPC), -1, np.int64)   # row -> global node id
    row_of = np.full(N, -1, np.int64)             # node -> row in its core
    RL = np.zeros(BLOCKS, np.int64)
    RH = np.zeros(BLOCKS, np.int64)
    for c in range(NC):
        nodes = np.where(core_of == c)[0]
        for cls, row0 in ((True, 0), (False, ROWS_A)):
            ids = nodes[is_a_node[nodes] == cls]
            o = ids[np.lexsort((-(ka_n[ids] + kb_n[ids]),
                                -np.maximum(ka_n[ids], kb_n[ids])))]
            nodes_of[c, row0:row0 + len(o)] = o
            row_of[o] = row0 + np.arange(len(o))
        kar = np.where(nodes_of[c] >= 0, ka_n[np.maximum(nodes_of[c], 0)], 0)
        kbr = np.where(nodes_of[c] >= 0, kb_n[np.maximum(nodes_of[c], 0)], 0)
        RL = np.maximum(RL, kar.reshape(BLOCKS, 128).max(axis=1))
        RH = np.maximum(RH, kbr.reshape(BLOCKS, 128).max(axis=1))

    src_core = core_of[src]
    src_row = row_of[src]
    e_is_a = src_row < ROWS_A
    rows_a_all = src_core * ROWS_A + src_row
    rows_b_all = src_core * ROWS_B + (src_row - ROWS_A)
    slot_all = row_of[dst]

    idx_a_cores, idx_b_cores = [], []
    for c in range(NC):
        m = core_of[dst] == c
        slot_of = slot_all[m]
        is_a = e_is_a[m]
        rows_a = rows_a_all[m]
        rows_b = rows_b_all[m]
        la, lb = [], []
        for bidx in range(BLOCKS):
            base = bidx * 128
            in_blk = (slot_of >= base) & (slot_of < base + 128)
            sel = in_blk & is_a
            la.append(_fill_grid(RL[bidx], slot_of[sel] - base, rows_a[sel],
                                 DUMMY_A))
            sel = in_blk & ~is_a
            lb.append(_fill_grid(RH[bidx], slot_of[sel] - base, rows_b[sel],
                                 DUMMY_B))
        idx_a_cores.append(np.concatenate(la).astype(np.int16))
        idx_b_cores.append(np.concatenate(lb).astype(np.int16))

    return nodes_of, idx_a_cores, idx_b_cores, RL, RH


def _alpha_mask():
    """[128, BLOCKS] f32: -1e30 on pad rows (3199, 6251..6271), else 0."""
    mask = np.zeros((NPC,), np.float32)
    mask[PAD_A] = ALPHA_NEG
    mask[6251:] = ALPHA_NEG
    return np.ascontiguousarray(mask.reshape(BLOCKS, 128).T)


def _wrap_idx(flat):
    """[num] -> [128, num//16] wrapped (i%16, i//16), replicated to 128 parts."""
    num = len(flat)
    assert num % 16 == 0
    w = flat.reshape(num // 16, 16).T
    return np.ascontiguousarray(np.tile(w, (8, 1))).astype(np.int16)


# ---------------------------------------------------------------------------
# Device kernel builder
# ---------------------------------------------------------------------------

def _gather_sbuf(nc, out_ap, in_ap, idxs_ap, num_idxs, elem_size, queue_num,
                 reg=None):
    """Non-transpose dma_gather from an SBUF-resident table.

    Mirrors concourse.bass.BassGpSimd.dma_gather minus its "SBUF source
    implies transpose" restriction: the Q7 ucode's SBUF addressing branch
    (token = idx % 128 -> partition, rank = idx // 128 -> free-dim stripe)
    is independent of the transpose flag, and the non-transpose RX side
    writes the standard [128, num_idxs/128, elem] grid layout.
    """
    import concourse.mybir as mybir

    eng = nc.gpsimd
    elem_bytes = elem_size * mybir.dt.size(in_ap.dtype)
    return eng.add_instruction(
        mybir.InstDMAGatherAnt(
            name=eng.bass.get_next_instruction_name(),
            ins=[
                eng.lower_ap(in_ap),
                eng.lower_ap(idxs_ap),
                eng.lower_val_access(reg if reg is not None
                                     else eng.to_reg(num_idxs)),
            ],
            outs=[eng.lower_ap(out_ap)],
            transpose=False,
            num_idxs=num_idxs,
            elem_size=elem_size,
            stride_bytes_256=0,
            gen_mode=0,
            single_packet=True,
            queue_num=queue_num,
            sbuf_tokens_per_rank=128,
            sbuf_free_dim_per_rank=elem_bytes,
            sbuf_free_dim_pad_per_rank=0,
            sbuf_byte_offset=0,
        )
    )


def _build(nc, RL, RH, n_idx_a, n_idx_b):
    import contextlib

    import concourse.mybir as mybir
    import concourse.tile as tile
    from concourse import library_config
    from concourse.masks import make_identity

    f32 = mybir.dt.float32
    f16 = mybir.dt.float16
    AF = mybir.ActivationFunctionType
    ALU = mybir.AluOpType

    # --- I/O ---
    # x is pre-transposed on the host so layer-0 projection feeds the PE
    # stationary operand straight from DRAM (no per-block PE transpose).
    x_in = nc.dram_tensor("xT_own", [F_IN, NPC], f16, kind="ExternalInput").ap()
    w1_in = nc.dram_tensor("w1", [F_IN, H], f32, kind="ExternalInput").ap()
    w23_in = nc.dram_tensor("w23", [L - 1, H, H], f32, kind="ExternalInput").ap()
    asrc_in = nc.dram_tensor("asrc", [L, H], f32, kind="ExternalInput").ap()
    adst_in = nc.dram_tensor("adst", [L, H], f32, kind="ExternalInput").ap()
    bias_in = nc.dram_tensor("bias", [L, H], f32, kind="ExternalInput").ap()
    wout_in = nc.dram_tensor("wout", [H, OUT], f32, kind="ExternalInput").ap()
    bout_in = nc.dram_tensor("bout", [1, OUT], f32, kind="ExternalInput").ap()
    idxa_in = nc.dram_tensor("idx_a", [128, n_idx_a // 16], mybir.dt.int16,
                             kind="ExternalInput").ap()
    idxb_in = nc.dram_tensor("idx_b", [128, n_idx_b // 16], mybir.dt.int16,
                             kind="ExternalInput").ap()
    amask_in = nc.dram_tensor("alpha_mask", [128, BLOCKS], f32,
                              kind="ExternalInput").ap()
    out_t = nc.dram_tensor("y", [NPC, OUT], f32, kind="ExternalOutput").ap()

    # --- internal DRAM ---
    # Compact partition-major tables: core-local row r lives at
    # [r % 128, r // 128, :], so the post-AllGather DRAM->SBUF fill runs at
    # line rate (one big descriptor per (core, partition)).
    BLOCKS_B = BLOCKS - BLOCKS_A
    # The table halves are AllGathered in four rank-range splits so each
    # collective fires as soon as its projections land; only the last split's
    # (short) chain remains exposed at a layer boundary.
    # (class, name, rank lo, rank hi) -- ranks are within the class table.
    TAB_SPLITS = [("a", "a1", 0, 10), ("a", "a2", 10, BLOCKS_A),
                  ("b", "b1", 0, 12), ("b", "b2", 12, BLOCKS_B)]
    tab_own = {}
    tab_full = {}
    for cls, nm, lo, hi in TAB_SPLITS:
        tab_own[nm] = nc.dram_tensor(f"tab_own_{nm}", [128, hi - lo, ELEM],
                                     f32, kind="Internal").ap()
        tab_full[nm] = nc.dram_tensor(f"tab_full_{nm}",
                                      [NC, 128, hi - lo, ELEM], f32,
                                      kind="Internal",
                                      addr_space="Shared").ap()

    R_TOT = [int(RL[b] + RH[b]) for b in range(BLOCKS)]
    R_MAX = max(R_TOT)
    CHUNK = 1024         # gather chunk (descriptors per SWDGE instruction)
    LOOKAHEAD = 10       # blocks the A-class gather stream leads by
    R_CAP = 38           # max rounds per multiply+tree segment (wt scratch)
    GA, GB = 5, 6        # row-store group sizes (25 = 5*5, 24 = 4*6)
    PGRP = 8             # next-layer projection burst size
    PGRP_Y = 8           # last-layer output projection burst size

    # Per-block queue: all of a block's gather chunks share one queue so a
    # blocked chunk (grid-pool WAR, pending fill) parks only its own queue.
    # Greedy-balance block costs (~R_TOT) across the 4 SWDGE queues.
    qload = [0] * 4
    QMAP = [0] * BLOCKS
    for b in sorted(range(BLOCKS), key=lambda x: -R_TOT[x]):
        q = min(range(4), key=lambda i: qload[i])
        QMAP[b] = q
        qload[q] += R_TOT[b]

    warm_in = nc.dram_tensor("cc_warm_in", [1, 16], f32, kind="Internal").ap()
    warm_out = nc.dram_tensor("cc_warm_out", [NC, 1, 16], f32, kind="Internal",
                              addr_space="Shared").ap()

    with tile.TileContext(nc) as tc:
        nc.gpsimd.load_library(library_config.mlp)

        with contextlib.ExitStack() as ctx:
            const = ctx.enter_context(tc.tile_pool(name="const", bufs=1))
            psum = ctx.enter_context(tc.tile_pool(name="psum", bufs=4, space="PSUM"))
            work = ctx.enter_context(tc.tile_pool(name="work", bufs=3))
            small = ctx.enter_context(tc.tile_pool(name="small", bufs=4))

            nc.gpsimd.collective_compute(
                "AllGather", ALU.bypass, replica_groups=[list(range(NC))],
                ins=[warm_in.opt()], outs=[warm_out.opt()])
            ident = const.tile([128, 128], f32, tag="ident")
            make_identity(nc, ident[:])
            ident16 = const.tile([128, 128], f16, tag="ident16")
            make_identity(nc, ident16[:])
            ones_row = const.tile([1, 128], f32, tag="ones")
            nc.vector.memset(ones_row[:], 1.0)
            idxa_sb = const.tile([128, n_idx_a // 16], mybir.dt.int16, tag="idxa")
            nc.sync.dma_start(idxa_sb[:], idxa_in[:])
            idxb_sb = const.tile([128, n_idx_b // 16], mybir.dt.int16, tag="idxb")
            nc.sync.dma_start(idxb_sb[:], idxb_in[:])
            x_buf = const.tile([128, BLOCKS * H], f16, tag="xbuf")
            jk_buf = const.tile([128, BLOCKS * H], f16, tag="jkbuf")
            sb_tab_a = const.tile([128, NC * BLOCKS_A * ELEM], f32, tag="taba")
            sb_tab_b = const.tile([128, NC * BLOCKS_B * ELEM], f32, tag="tabb")
            sb_ta3 = sb_tab_a[:].rearrange("p (k e) -> p k e", e=ELEM)
            sb_tb3 = sb_tab_b[:].rearrange("p (k e) -> p k e", e=ELEM)
            alphad = const.tile([128, BLOCKS], f32, tag="alphad")
            alphas = const.tile([128, BLOCKS], f32, tag="alphas")
            h_buf = const.tile([128, BLOCKS * H], f16, tag="hbuf")
            amask = const.tile([128, BLOCKS], f32, tag="amask")
            nc.sync.dma_start(amask[:], amask_in[:])
            ebias = const.tile([128, 1], f32, tag="ebias")
            nc.vector.memset(ebias[:], -2.772588722239781)

            self_q = [0]
            stage_state = {}

            def prep_weights(layer):
                """[W | W@a_src | W@a_dst] + bias broadcast tile for layer."""
                F = F_IN if layer == 0 else H
                w_ap = w1_in if layer == 0 else w23_in[layer - 1]
                waug = small.tile([128, H + 2], f32, tag="waug")
                nc.sync.dma_start(waug[:F, 0:H], w_ap)
                wt_ps = psum.tile([H, 128], f32, tag="ps_t")
                nc.tensor.transpose(wt_ps[:, :F], waug[:F, 0:H], ident[:F, :F])
                wt_sb = small.tile([H, 128], f32, tag="wtsb")
                nc.scalar.copy(wt_sb[:, :F], wt_ps[:, :F])
                a_cols = small.tile([H, 2], f32, tag="acols")
                nc.sync.dma_start(a_cols[:, 0:1], asrc_in[layer, :, None])
                nc.sync.dma_start(a_cols[:, 1:2], adst_in[layer, :, None])
                va_ps = psum.tile([128, 2], f32, tag="ps_m")
                nc.tensor.matmul(va_ps[:F, :], wt_sb[:, :F], a_cols[:],
                                 start=True, stop=True)
                nc.vector.tensor_copy(waug[:F, H:H + 2], va_ps[:F, :])
                b_row = small.tile([1, H], f32, tag="brow")
                nc.sync.dma_start(b_row[:], bias_in[layer, None, :])
                bt_ps = psum.tile([128, H], f32, tag="ps_m")
                nc.tensor.matmul(bt_ps[:], ones_row[:], b_row[:],
                                 start=True, stop=True)
                b_tile = small.tile([128, H], f32, tag="btile")
                nc.scalar.copy(b_tile[:], bt_ps[:])
                waug16 = small.tile([128, H + 2], f16, tag="waug16")
                nc.scalar.copy(waug16[:F, :], waug[:F, :])
                return waug16, b_tile

            xg_bufs = [const.tile([F_IN, 7 * 128], f16, tag=f"xg{i}",
                                  name=f"xg{i}") for i in range(2)]
            xg_state = {}

            def proj_block(layer, t, waug):
                """Project block t of `layer`, stage the packed 136B table
                rows, flush per group, and trigger the half-AllGathers."""
                F = F_IN if layer == 0 else H
                if layer == 0:
                    # batched x loads: one DMA per 8 blocks keeps the sync
                    # queue short so the staging flushes aren't delayed
                    if t % 7 == 0:
                        xg_state[0] = xg_bufs[(t // 7) % 2]
                        hi = min((t + 7) * 128, NPC)
                        nc.sync.dma_start(xg_state[0][:, 0:hi - t * 128],
                                          x_in[:, t * 128:hi])
                    xT_sb = xg_state[0][:, (t % 7) * 128:(t % 7 + 1) * 128]
                else:
                    xt = x_buf[:, t * H:(t + 1) * H]
                    xT_ps = psum.tile([H, 128], f16, tag="ps_t")
                    nc.tensor.transpose(xT_ps[:], xt, ident16[:])
                    xT_sb = work.tile([H, 128], f16, tag="xTsb")
                    nc.scalar.copy(xT_sb[:], xT_ps[:])
                h_ps = psum.tile([128, H + 2], f32, tag="ps_m")
                xT_ap = xT_sb if layer == 0 else xT_sb[:]
                nc.tensor.matmul(h_ps[:], xT_ap, waug[:F, :],
                                 start=True, stop=True)
                # group staging (partition-major compact rows)
                G = GA if t < BLOCKS_A else GB
                t0 = t if t < BLOCKS_A else t - BLOCKS_A
                if t0 % G == 0:
                    stage_state[layer] = work.tile([128, G * ELEM], f32,
                                                   tag="rowstg",
                                                   name="rowstg")
                stg = stage_state[layer]
                j = t0 % G
                stg16 = stg[:].bitcast(f16)
                (nc.vector.tensor_copy if layer == 0 else nc.scalar.copy)(
                    stg16[:, j * 2 * ELEM:j * 2 * ELEM + H],
                    h_ps[:, 0:H])
                nc.scalar.activation(stg[:, j * ELEM + 32:j * ELEM + 33],
                                     h_ps[:, H:H + 1], AF.Identity,
                                     bias=amask[:, t:t + 1])
                nc.vector.tensor_copy(alphad[:, t:t + 1], h_ps[:, H + 1:H + 2])
                nc.vector.tensor_copy(alphas[:, t:t + 1], h_ps[:, H:H + 1])
                nc.vector.tensor_copy(h_buf[:, t * H:(t + 1) * H],
                                      h_ps[:, 0:H])
                if j == G - 1:
                    cls = "a" if t < BLOCKS_A else "b"
                    for c2, nm, lo, hi in TAB_SPLITS:
                        if c2 == cls and lo <= t0 - j and t0 < hi:
                            nc.sync.dma_start(
                                tab_own[nm][:, t0 - j - lo:t0 + 1 - lo, :],
                                stg[:].rearrange("p (g e) -> p g e", e=ELEM))


            def fill_split(split, eng=None):
                cls, nm, lo, hi = split
                tgt, nblk = ((sb_ta3, BLOCKS_A) if cls == "a"
                             else (sb_tb3, BLOCKS_B))
                for c in range(NC):
                    (eng or nc.sync).dma_start(
                        tgt[:, c * nblk + lo:c * nblk + hi, :],
                        tab_full[nm][c])

            def fills(skip_b2=False):
                for split in TAB_SPLITS:
                    if skip_b2 and split[1] == "b2":
                        continue
                    fill_split(split)

            offs_a = np.concatenate([[0], np.cumsum(128 * RL)]).astype(int)
            offs_b = np.concatenate([[0], np.cumsum(128 * RH)]).astype(int)

            # Per-class grid rings: A-class gathers run LOOKAHEAD blocks ahead
            # of the B-class + compute stream, so at a layer boundary the
            # queues hold W blocks of A-work while the B-half AllGather+fill
            # of the new layer completes.  Bump-allocated block offsets into
            # one const tile per class (same offsets every layer).
            def ring_offsets(sizes, window):
                cap = max(sum(sizes[m:m + window + 1])
                          for m in range(len(sizes))) + max(sizes)
                offs = []
                cur = 0
                for b, s in enumerate(sizes):
                    if cur + s > cap:
                        cur = 0
                    for j in range(max(0, b - window), b):
                        assert (cur + s <= offs[j]
                                or cur >= offs[j] + sizes[j]), (b, j)
                    offs.append(cur)
                    cur += s
                return offs, cap

            sizes_a = [int(RL[b]) * ELEM for b in range(BLOCKS)]
            sizes_b = [int(RH[b]) * ELEM for b in range(BLOCKS)]
            offA, CAP_A = ring_offsets(sizes_a, LOOKAHEAD + 2)
            offB, CAP_B = ring_offsets(sizes_b, 5)
            gridA_buf = const.tile([128, CAP_A], f32, tag="gridA")
            gridB_buf = const.tile([128, CAP_B], f32, tag="gridB")

            def grid_view(buf, off, rounds):
                return buf[:, off:off + rounds * ELEM].rearrange(
                    "p (r h) -> p r h", h=ELEM)

            def build_chunks(sizes, offs_ring, offs_idx):
                """Fuse each class's per-block gathers into CHUNK-slot
                instructions spanning consecutive blocks (their ring regions
                are bump-adjacent), splitting at ring wraps.  Returns
                {flush_block: [(idx_off, ring_off, n_slots), ...]}."""
                out = {}
                pend = []          # (block, idx_off, ring_off, n_slots)
                pn = 0

                def flush():
                    nonlocal pend, pn
                    if not pn:
                        return
                    # keyed by FIRST covered block: the chunk must be issued
                    # before that block's edge_compute
                    out.setdefault(pend[0][0], []).append(
                        (pend[0][1], pend[0][2], pn))
                    pend = []
                    pn = 0

                for b in range(BLOCKS):
                    n = sizes[b] // ELEM * 128
                    if pend and offs_ring[b] == 0:
                        flush()        # ring wrapped before this block
                    done = 0
                    while done < n:
                        take = min(CHUNK - pn, n - done)
                        pend.append((b, offs_idx[b] + done,
                                     offs_ring[b] + done // 128 * ELEM, take))
                        pn += take
                        done += take
                        if pn == CHUNK:
                            flush()
                    if b == BLOCKS - 1:
                        flush()
                return out

            chunksA = build_chunks(sizes_a, offA, [int(v) for v in offs_a])
            chunksB = build_chunks(sizes_b, offB, [int(v) for v in offs_b])

            def issue_chunks(chunks, b, buf, isb):
                for idx_off, ring_off, n in chunks.get(b, ()):
                    _gather_sbuf(
                        nc,
                        buf[:, ring_off:ring_off + (n // 128) * ELEM]
                        .rearrange("p (r h) -> p r h", h=ELEM),
                        sb_tab_a[:] if buf is gridA_buf else sb_tab_b[:],
                        isb[:, idx_off // 16:(idx_off + n) // 16],
                        n, ELEM,
                        queue_num=self_q[0] % 4,
                    )
                    self_q[0] += 1

            def edge_compute(layer, b, b_tile):
                rl, rh, rt = int(RL[b]), int(RH[b]), R_TOT[b]
                grA = grid_view(gridA_buf, offA[b], rl)
                grB = grid_view(gridB_buf, offB[b], rh)
                tbuf = work.tile([128, R_MAX], f32, tag="tbuf")
                nc.scalar.activation(tbuf[:, 0:rl], grA[:, 0:rl, 32],
                                     AF.Identity, bias=alphad[:, b:b + 1])
                nc.scalar.activation(tbuf[:, rl:rt], grB[:, 0:rh, 32],
                                     AF.Identity, bias=alphad[:, b:b + 1])
                nc.vector.scalar_tensor_tensor(
                    out=tbuf[:, 0:rt], in0=tbuf[:, 0:rt],
                    scalar=NEG_SLOPE, in1=tbuf[:, 0:rt],
                    op0=ALU.mult, op1=ALU.max)
                p_t = work.tile([128, R_MAX], f16, tag="ptile")
                den = small.tile([128, 1], f32, tag="den")
                nc.scalar.activation(p_t[:, 0:rt], tbuf[:, 0:rt], AF.Exp,
                                     bias=ebias[:, 0:1], accum_out=den[:])
                hA = (gridA_buf[:, offA[b]:offA[b] + rl * ELEM].bitcast(f16)
                      .rearrange("p (r h) -> p r h", h=2 * ELEM)[:, :, 0:H])
                hB = (gridB_buf[:, offB[b]:offB[b] + rh * ELEM].bitcast(f16)
                      .rearrange("p (r h) -> p r h", h=2 * ELEM)[:, :, 0:H])

                # weighted multiply + halving-tree reduction, in segments of
                # at most R_CAP rounds so the wt scratch stays small (only
                # blocks 0 and 25 exceed R_CAP); contiguous in-place f16
                # adds replace the old strided (transposed) reduce_sum
                num_t = work.tile([128, H], f32, tag="num")
                wt = work.tile([128, H * R_CAP], f16, tag="wtile")
                wt3 = wt[:].rearrange("p (r f) -> p r f", f=H)

                def seg_mult(r0, r1):
                    """wt3[0:r1-r0] = h rows (A/B concat) * p for rounds
                    [r0, r1)."""
                    n_a = max(0, min(rl, r1) - r0)
                    if n_a > 0:
                        nc.vector.tensor_tensor(
                            out=wt3[:, 0:n_a, :], in0=hA[:, r0:r0 + n_a, :],
                            in1=p_t[:, r0:r0 + n_a].unsqueeze(2)
                            .to_broadcast([128, n_a, H]), op=ALU.mult)
                    n_b = r1 - r0 - n_a
                    if n_b > 0:
                        b0 = max(0, r0 - rl)
                        nc.vector.tensor_tensor(
                            out=wt3[:, n_a:n_a + n_b, :],
                            in0=hB[:, b0:b0 + n_b, :],
                            in1=p_t[:, r0 + n_a:r1].unsqueeze(2)
                            .to_broadcast([128, n_b, H]), op=ALU.mult)

                nseg = (rt + R_CAP - 1) // R_CAP
                for s in range(nseg):
                    r0, r1 = s * R_CAP, min((s + 1) * R_CAP, rt)
                    seg_mult(r0, r1)
                    m = r1 - r0
                    while m > 2:
                        h2 = m // 2
                        nc.vector.tensor_tensor(
                            out=wt3[:, 0:h2, :], in0=wt3[:, 0:h2, :],
                            in1=wt3[:, m - h2:m, :], op=ALU.add)
                        m = h2 + (m & 1)
                    if s == 0:
                        nc.vector.tensor_tensor(
                            out=num_t[:], in0=wt3[:, 0, :], in1=wt3[:, 1, :],
                            op=ALU.add)
                    else:
                        nc.vector.tensor_tensor(
                            out=num_t[:], in0=num_t[:], in1=wt3[:, 0, :],
                            op=ALU.add)
                        nc.vector.tensor_tensor(
                            out=num_t[:], in0=num_t[:], in1=wt3[:, 1, :],
                            op=ALU.add)
                # self-loop handled locally: p_self = exp(lrelu(as+ad) - C)
                # (tbuf is free after the exp; reuse two of its columns)
                zs = tbuf[:, 0:1]
                ps_self = tbuf[:, 1:2]
                nc.vector.tensor_tensor(out=zs, in0=alphas[:, b:b + 1],
                                        in1=alphad[:, b:b + 1], op=ALU.add)
                nc.vector.scalar_tensor_tensor(
                    out=zs, in0=zs, scalar=NEG_SLOPE, in1=zs,
                    op0=ALU.mult, op1=ALU.max)
                nc.scalar.activation(ps_self, zs, AF.Exp,
                                     bias=ebias[:, 0:1])
                nc.vector.tensor_tensor(out=den[:], in0=den[:],
                                        in1=ps_self, op=ALU.add)
                nc.vector.scalar_tensor_tensor(
                    out=num_t[:], in0=h_buf[:, b * H:(b + 1) * H],
                    scalar=ps_self, in1=num_t[:],
                    op0=ALU.mult, op1=ALU.add)
                num = num_t[:]
                nc.vector.tensor_scalar_max(den[:], den[:], 1e-30)
                recip = small.tile([128, 1], f32, tag="recip")
                nc.vector.reciprocal(recip[:], den[:])
                jk = jk_buf[:, b * H:(b + 1) * H]
                if layer < L - 1:
                    xn = x_buf[:, b * H:(b + 1) * H]
                    nc.vector.scalar_tensor_tensor(
                        out=xn, in0=num, scalar=recip[:, 0:1],
                        in1=b_tile[:], op0=ALU.mult, op1=ALU.add)
                    nc.scalar.activation(xn, xn, AF.Relu)
                    if layer == 0:
                        nc.scalar.copy(jk, xn)
                    else:
                        nc.vector.tensor_tensor(out=jk, in0=jk, in1=xn,
                                                op=ALU.max)
                else:
                    xn = work.tile([128, H], f16, tag="xnlast",
                                   name="xnlast")[:]
                    nc.vector.scalar_tensor_tensor(
                        out=xn, in0=num, scalar=recip[:, 0:1],
                        in1=b_tile[:], op0=ALU.mult, op1=ALU.add)
                    nc.vector.scalar_tensor_tensor(
                        out=jk, in0=xn, scalar=0.0, in1=jk,
                        op0=ALU.max, op1=ALU.max)

            def y_proj(t, wout_sb, bo_tile):
                jt = jk_buf[:, t * H:(t + 1) * H]
                jT_ps = psum.tile([H, 128], f16, tag="ps_t")
                nc.tensor.transpose(jT_ps[:], jt, ident16[:])
                jT_sb = work.tile([H, 128], f16, tag="jTsb")
                nc.scalar.copy(jT_sb[:], jT_ps[:])
                y_ps = psum.tile([128, OUT], f32, tag="ps_m")
                nc.tensor.matmul(y_ps[:], jT_sb[:], wout_sb[:],
                                 start=True, stop=True)
                y_sb = work.tile([128, OUT], f32, tag="ysb")
                nc.vector.tensor_tensor(out=y_sb[:], in0=y_ps[:],
                                        in1=bo_tile[:], op=ALU.add)
                nc.sync.dma_start(out_t[t * 128:(t + 1) * 128, :], y_sb[:])

            def fire_ag(nm):
                nc.gpsimd.collective_compute(
                    "AllGather", ALU.bypass,
                    replica_groups=[list(range(NC))],
                    ins=[tab_own[nm].opt()], outs=[tab_full[nm].opt()])

            # proj step after which each split's staging has fully landed
            AG_AT = {9: "a1", 24: "a2", 36: "b1", 48: "b2"}

            # ---- layer 0 projection (x from DRAM) ----
            waug, b_tile = prep_weights(0)
            for t in range(BLOCKS):
                proj_block(0, t, waug)
                if t in AG_AT:
                    fire_ag(AG_AT[t])
            fills()

            # ---- layers ----
            for layer in range(L):
                if layer < L - 1:
                    waug_n, b_tile_n = prep_weights(layer + 1)
                else:
                    wout_f32 = const.tile([H, OUT], f32, tag="woutf32")
                    nc.sync.dma_start(wout_f32[:], wout_in[:])
                    wout_sb = const.tile([H, OUT], f16, tag="wout")
                    nc.scalar.copy(wout_sb[:], wout_f32[:])
                    bo_row = const.tile([1, OUT], f32, tag="borow")
                    nc.sync.dma_start(bo_row[:], bout_in[:])
                    bo_ps = psum.tile([128, OUT], f32, tag="ps_m")
                    nc.tensor.matmul(bo_ps[:], ones_row[:], bo_row[:],
                                     start=True, stop=True)
                    bo_tile = const.tile([128, OUT], f32, tag="botile")
                    nc.scalar.copy(bo_tile[:], bo_ps[:])

                for i in range(BLOCKS + LOOKAHEAD):
                    if i < BLOCKS:
                        issue_chunks(chunksA, i, gridA_buf, idxa_sb)
                    if i == 0 and layer > 0:
                        fill_split(("b", "b1", 0, 12))
                    if i == 6 and layer > 0:
                        # deferred b2 AllGather + fill of THIS layer's table:
                        # placed after a few A-gather issues so the new
                        # layer's A stream dispatches ahead of the trigger's
                        # wait (the fill must FOLLOW the trigger in program
                        # order to bind to this layer's AllGather)
                        fire_ag("b2")
                        fill_split(("b", "b2", 12, BLOCKS_B))
                    if i >= LOOKAHEAD:
                        b = i - LOOKAHEAD
                        issue_chunks(chunksB, b, gridB_buf, idxb_sb)
                        edge_compute(layer, b, b_tile)
                        # burst the next layer's projection every PGRP blocks
                        # to keep its PE->ACT round trips off the per-block
                        # chain while still firing the AllGathers mid-stream
                        grp = PGRP if layer < L - 1 else PGRP_Y
                        if b % grp == grp - 1 or b == BLOCKS - 1:
                            for t in range(b - b % grp, b + 1):
                                if layer < L - 1:
                                    proj_block(layer + 1, t, waug_n)
                                    if t in AG_AT and AG_AT[t] != "b2":
                                        fire_ag(AG_AT[t])
                                else:
                                    y_proj(t, wout_sb, bo_tile)
                        # next layer's a-fills early: the AGs are long done,
                        # and this keeps them off the SP queue tail where
                        # they'd sit behind the last staging flush
                        if b == 44 and layer < L - 1:
                            fill_split(("a", "a1", 0, 10))
                            fill_split(("a", "a2", 10, BLOCKS_A))
                if layer < L - 1:
                    waug, b_tile = waug_n, b_tile_n

    return nc


# ---------------------------------------------------------------------------
# Entry point
# ---------------------------------------------------------------------------

def kernel(x, edge_index, W1, W23, a_src, a_dst, b, Wout, bout):
    import concourse.bacc as bacc
    from concourse import bass_utils

    x = np.asarray(x, np.float32)
    edge_index = np.asarray(edge_index)
    nodes_of, idx_a, idx_b, RL, RH = _preprocess(edge_index.astype(np.int64))

    n_idx_a = len(idx_a[0])
    n_idx_b = len(idx_b[0])

    nc = bacc.Bacc("TRN2", target_bir_lowering=False, debug=False, num_devices=NC,
                   num_swdge_queues=4, dynamic_dma_scratch_size=24576)
    _build(nc, RL, RH, n_idx_a, n_idx_b)
    nc.compile()

    in_maps = []
    for c in range(NC):
        nodes = nodes_of[c]
        x_own = np.zeros((NPC, F_IN), np.float32)
        valid = np.nonzero(nodes >= 0)[0]
        x_own[valid] = x[nodes[valid]]
        in_maps.append({
            "xT_own": np.ascontiguousarray(x_own.T.astype(np.float16)),
            "w1": np.asarray(W1, np.float32),
            "w23": np.asarray(W23, np.float32),
            "asrc": np.asarray(a_src, np.float32),
            "adst": np.asarray(a_dst, np.float32),
            "bias": np.asarray(b, np.float32),
            "wout": np.asarray(Wout, np.float32),
            "bout": np.asarray(bout, np.float32).reshape(1, OUT),
            "idx_a": _wrap_idx(idx_a[c]),
            "idx_b": _wrap_idx(idx_b[c]),
            "alpha_mask": _alpha_mask(),
        })

    res = bass_utils.run_bass_kernel_spmd(nc, in_maps, core_ids=list(range(NC)))
    global _last_results
    _last_results = res
    out = np.zeros((N, OUT), np.float32)
    for c in range(NC):
        y = res.results[c]["y"]
        nodes = nodes_of[c]
        valid = np.nonzero(nodes >= 0)[0]
        out[nodes[valid]] = y[valid]
    return out

